# revision 65
# baseline (speedup 1.0000x reference)
"""Trainium2 Bass kernel for nn_GatedFeedForward (gated feed-forward with
feature attention).

Reference computation per batch b (B=8, N=4096, D=1024):
    VR = x @ Wvr.T ; VI = x @ Wvi.T            (biases are zero)
    V  = VR * tanh(softplus(VI))
    K  = x @ Wk.T  ; Q  = x @ Wq.T
    Kn = K / (||K||_col + 1e-5) ; Qn = Q / (||Q||_col + 1e-5)   (norm over N)
    A  = smu(Kn.T @ Qn)     # == leaky-relu slope 0.25 == 0.625x + 0.375|x|
    out = V @ A
Sharding: pure data-parallel over batch — one batch per NeuronCore.

Key algebraic restructure: with S = X^T X (D x D, one N-contraction),
    K^T Q        = WkT^T S WqT          (WkT = Wk.T, [in,out])
    ||K_d||^2    = colsum(WkT * (S WkT))
    ||Q_e||^2    = colsum(WqT * (S WqT))
so the K/Q path costs ~0.56 N*D^2 (S upper triangle) plus three D^3
matmuls instead of 3 N*D^2. leaky's positive homogeneity folds rk into
A's rows and rq into the output tiles.

Precision: bf16 matmuls with fp32 PSUM, EXCEPT fp8-e4m3 DoubleRow
(2x PE rate, both operands fp8, 256-deep contraction per instruction)
for three noise-tolerant contractions (tolerance 2e-2, measured
1.89e-2, all verified against a numpy e4m3 simulation first):
  - VI (gate input): only passes through the saturating gate
    tanh(softplus(.)), so ~5% fp8 noise -> ~1.5e-2 output noise.
  - Uq = S WqT: feeds only nq2 = colsum(WqT*Uq), a sum of squares
    where fp8 noise averages to ~0.1% on rq.
  - ~31% of S's blocks (6 of 20 groups + mirrors): each fraction f of
    S in fp8 adds sqrt(f)*1.7e-2 output noise.
Operand pow2 scales (W*4096, x*32, S*2^-5) are folded into the gate
activation constants / rsqrt scale / PSUM->bf16 cast scales.

The gate tanh(softplus(x)) is evaluated as c0 + c1*tanh(a1 x + b1)
+ c2*tanh(a2 x + b2) (max abs err 3.9e-3): both ops hit the resident
tanh activation table — no table switches, no slow DVE reciprocal.

Pass-1 S uses 256-wide column spans (triangle = 20 of 32 blocks
instead of 12 of 16 at 512-wide). Two [P,256] accumulation groups
share each 2 KB PSUM bank; since start_tensor_calc zeroes the WHOLE
bank, only the temporally-first matmul of each bank carries start=True
and the partner group's first matmul lands on pending-zero bytes.

Schedule per core (pass 1 is right at the DMA roofline — the xn
stream crosses HBM twice — so scheduling there is DMA-first):
  Pass 1: S in two column superphases (cols 512:1024 = 12 groups/6
      banks, then cols 0:512 = 8 groups/4 banks at +2 bank offset so
      phase B never waits on phase A's last casts), accumulated in
      PSUM across all 8 sequence chunks; cast to bf16 (scaled for the
      fp8 groups) at superphase end. The lower-triangle transpose
      reconstruction and the Tk columns 6,7 (which depend only on
      phase-A blocks) interleave into phase B's DMA-paced stream;
      fp8 groups run last within each chunk so their operand casts
      (DVE/ACT) stay ahead of the PE. wk/wq trickle one tile per
      queue per chunk-set; wvr trickles through the mid G loop.
  Mid:    Tk (+nk2 via per-block ones-matmuls straight into column
      layout), Uq in fp8 (+nq2), G -> A' from PSUM; S->fp8 pair casts
      ride the Tk iterations.
  Pass 2: VRVI(c) / out(c) software-pipelined with lag 1 so the PE
      never waits on the gate chain; xt/xt8 split across sync/gpsimd
      queues, prefetched 3 chunks ahead. Output stored bf16, upcast
      on host.
"""

import numpy as np
import ml_dtypes

import concourse.bass as bass
import concourse.tile as tile
from concourse import bacc, mybir
from concourse.bass import ts

F32 = mybir.dt.float32
BF16 = mybir.dt.bfloat16
FP8 = mybir.dt.float8e4
AF = mybir.ActivationFunctionType
ALU = mybir.AluOpType
DR = mybir.MatmulPerfMode.DoubleRow

B = 8
N_FULL = 4096
D_FULL = 1024
N_CORES = 8

P = 128  # SBUF partitions
NC = 512  # sequence chunk
EF = 512  # free-dim span per mid/out matmul / psum bank
ES = 256  # pass-1 S column span (two spans share a psum bank)

# fp8 operand scales (pow2; folded into gate constants)
SW = 4096.0  # Wvi/Wq scale: xavier max 0.0542*4096 = 222 < 240
SX = 32.0    # x scale: |x|max ~5.2*32 = 167 < 240
VI_SCL = SW * SX
SXS = 2.0 ** -5  # S scale: diag ~N=4096 -> 128 < 240
UQ_SCL = SW * SXS  # Uq psum carries Uq * 128

# tanh-sum fit of tanh(softplus(x)), max |err| 3.9e-3 on [-12, 12]
GC0, GC1, GA1, GB1 = 0.50022747, 0.32785149, 0.8261997, -0.02962021
GC2, GA2, GB2 = 0.17216236, 0.57575332, 0.75023909


def build_program(n=N_FULL, d=D_FULL):
    """Build the single-core SPMD Bass program for one [n, d] batch."""
    assert n % NC == 0 and d % P == 0
    n_chunks = n // NC
    n_sub = NC // P  # 128-row subtiles per chunk
    n_dblk = d // P  # feature blocks
    ef = min(EF, d)
    n_ef = d // ef
    n_es = d // ES  # pass-1 column spans
    n_pair = n_dblk // 2  # fp8 DoubleRow pair-blocks

    nc = bacc.Bacc("TRN2", target_bir_lowering=False, debug=False,
                   num_devices=N_CORES)
    ident = nc.dram_tensor("ident", [P, P], BF16, kind="ExternalInput")
    xn = nc.dram_tensor("xn", [n, d], BF16, kind="ExternalInput")
    xt = nc.dram_tensor("xt", [d, n], BF16, kind="ExternalInput")
    xt8 = nc.dram_tensor("xt8", [d, n], FP8, kind="ExternalInput")
    wvr = nc.dram_tensor("wvr", [d, d], BF16, kind="ExternalInput")
    wvi8 = nc.dram_tensor("wvi8", [d, d], FP8, kind="ExternalInput")
    wq8 = nc.dram_tensor("wq8", [d, d], FP8, kind="ExternalInput")
    wk = nc.dram_tensor("wk", [d, d], BF16, kind="ExternalInput")
    wq = nc.dram_tensor("wq", [d, d], BF16, kind="ExternalInput")
    out_d = nc.dram_tensor("out", [n, d], BF16, kind="ExternalOutput")

    with tile.TileContext(nc) as tc:
        with tc.tile_pool(name="const", bufs=1) as const_pool, \
             tc.tile_pool(name="w", bufs=1) as w_pool, \
             tc.tile_pool(name="sb", bufs=1) as sb_pool, \
             tc.tile_pool(name="tkb", bufs=1) as tkb_pool, \
             tc.tile_pool(name="post", bufs=1) as post_pool, \
             tc.tile_pool(name="apost", bufs=1) as ap_pool:
            # bf16 ones for the norm partition-reduce matmuls: fp32 moving
            # operands stream at half rate and add PE dtype-mode switches
            onesb = const_pool.tile([P, 1], BF16, name="onesb", tag="onesb")
            nc.vector.memset(onesb, 1.0)
            ones_row = const_pool.tile([1, P], F32, name="ones_row", tag="onesr")
            nc.vector.memset(ones_row, 1.0)
            gb1 = const_pool.tile([P, 1], F32, name="gb1", tag="gb1")
            nc.vector.memset(gb1, GB1)
            gb2 = const_pool.tile([P, 1], F32, name="gb2", tag="gb2")
            nc.vector.memset(gb2, GB2)
            ident_sb = const_pool.tile([P, P], BF16, name="ident", tag="ident")

            w_tiles = {}
            for wname in ("wk", "wq", "wvr"):
                w_tiles[wname] = [
                    w_pool.tile([P, d], BF16, name=f"{wname}{db}", tag=f"{wname}{db}")
                    for db in range(n_dblk)
                ]
            # Wvi/Wq as fp8 pair-tiles: [:, i, :] holds W rows 256j+128i..+127
            wvi_tiles = [w_pool.tile([P, 2, d], FP8, name=f"wvi{j}", tag=f"wvi{j}")
                         for j in range(n_pair)]
            wq8_tiles = [w_pool.tile([P, 2, d], FP8, name=f"wq8{j}", tag=f"wq8{j}")
                         for j in range(n_pair)]
            # S as fp8 pair-tiles (cast from sb_tiles during Tk) for the Uq
            # DoubleRow matmuls
            s8_tiles = [w_pool.tile([P, 2, d], FP8, name=f"s8{j}", tag=f"s8{j}")
                        for j in range(n_pair)]

            sb_tiles = [sb_pool.tile([P, d], BF16, name=f"s{ib}", tag=f"s{ib}")
                        for ib in range(n_dblk)]
            tkb_tiles = [tkb_pool.tile([P, d], BF16, name=f"tk{ib}", tag=f"tk{ib}")
                         for ib in range(n_dblk)]
            a_tiles = [ap_pool.tile([P, d], BF16, name=f"a{db}", tag=f"a{db}")
                       for db in range(n_dblk)]
            acc_k = {e: post_pool.tile([P, ef], BF16, name=f"acck{e}",
                                       tag=f"acck{e}") for e in range(n_ef)}
            acc_q = {e: post_pool.tile([P, ef], BF16, name=f"accq{e}",
                                       tag=f"accq{e}") for e in range(n_ef)}

            def tk_unit(ib, e, pt, first, tmp_pool):
                """One Tk output block: Tk[:, ib-block x e-span] = S @ WkT,
                bf16 cast to tkb_tiles, and the nk2 partial product
                acc_k[e] (+)= WkT * Tk."""
                for jb in range(n_dblk):
                    nc.tensor.matmul(pt, lhsT=sb_tiles[jb][:, ts(ib, P)],
                                     rhs=w_tiles["wk"][jb][:, ts(e, ef)],
                                     start=(jb == 0), stop=(jb == n_dblk - 1))
                # cast on ACT (idle here) to keep the DVE under the matmul
                # pace with the accumulate added
                nc.scalar.activation(out=tkb_tiles[ib][:, ts(e, ef)],
                                     in_=pt, func=AF.Copy)
                tmp = tmp_pool.tile([P, ef], BF16, name="tmp", tag="wtmpb")
                nc.vector.tensor_mul(out=tmp, in0=pt,
                                     in1=w_tiles["wk"][ib][:, ts(e, ef)])
                if first:
                    nc.vector.tensor_copy(out=acc_k[e], in_=tmp)
                else:
                    nc.vector.tensor_add(out=acc_k[e], in0=acc_k[e], in1=tmp)

            # ---------------- Pass 1: S = X^T X, PSUM-resident ----------------
            # Upper triangle at [128-row x 256-col] granularity:
            #   col-span e (256 wide) needs row-blocks ib with 128*ib <= 256e+255
            # Two superphases, split 12/8 groups (6/4 banks) so two banks stay
            # free for the transpose scratch that interleaves with phase B.
            tri = [(e, ib) for e in range(n_es - 1, -1, -1)
                   for ib in range(min(n_dblk, (ES * (e + 1)) // P))]
            spa, spb = tri[:12], tri[12:]
            # ~31% of S (incl. mirrored blocks) in fp8 DoubleRow: adds
            # ~1e-2 quadrature noise (sim: 1.64e-2 -> 1.88e-2, limit 2e-2)
            # and halves those groups' PE time
            S_FP8_GROUPS = {(3, 0), (3, 1), (3, 2), (3, 3), (0, 0), (0, 1)}
            # groups (e, 2e+1) straddle the diagonal: compute only their
            # upper 128-col half, the lower half comes from a transpose
            HALF_GROUPS = {(e, 2 * e + 1) for e in range(n_es)}

            with tc.tile_pool(name="xn1", bufs=4) as xn_pool, \
                 tc.tile_pool(name="xn8", bufs=2) as xn8_pool, \
                 tc.tile_pool(name="s_ps", bufs=1, space="PSUM") as s_ps:

                # at kernel start the serial per-queue transfer stream paces
                # the PE: both hwdge queues carry xn throughout pass 1, and
                # the first chunk-set is halved across them per subtile
                dma_state = {"set": 0}

                # superphase B touches only columns < 768 (rhs spans e<=2,
                # lhsT blocks ib<=5), so its re-stream loads 3/4 width
                wb = ES * (max(e for e, ib in spb) + 1)

                def dma_xn(c):
                    iset = dma_state["set"]
                    dma_state["set"] += 1
                    w = d if iset < n_chunks else wb
                    tiles = []
                    for s in range(n_sub):
                        t = xn_pool.tile([P, d], BF16, name=f"xn{s}", tag=f"xn{s}")
                        if iset < 2:
                            nc.sync.dma_start(
                                out=t[:, 0:w // 2],
                                in_=xn[c * NC + s * P:c * NC + (s + 1) * P,
                                       0:w // 2])
                            nc.scalar.dma_start(
                                out=t[:, w // 2:w],
                                in_=xn[c * NC + s * P:c * NC + (s + 1) * P,
                                       w // 2:w])
                        else:
                            eng = nc.sync if (s + iset) % 2 == 0 else nc.scalar
                            eng.dma_start(
                                out=t[:, 0:w],
                                in_=xn[c * NC + s * P:c * NC + (s + 1) * P, 0:w])
                        tiles.append(t)
                    # fp8 pair casts (x * SX) for the DoubleRow S groups; the
                    # psum then carries S * SX^2, divided out at the cast
                    p8s = []
                    for sp in range(n_sub // 2):
                        t8 = xn8_pool.tile([P, 2, d], FP8, name=f"xn8_{sp}",
                                           tag=f"xn8_{sp}")
                        for i in range(2):
                            if (sp + i) % 2 == 0:
                                nc.vector.tensor_scalar_mul(
                                    out=t8[:, i, 0:w], in0=tiles[2 * sp + i][:, 0:w],
                                    scalar1=SX)
                            else:
                                nc.scalar.activation(
                                    out=t8[:, i, 0:w], in_=tiles[2 * sp + i][:, 0:w],
                                    func=AF.Copy, scale=SX)
                        p8s.append(t8)
                    return tiles, p8s

                # lower-left S blocks = transposes of the computed upper-right
                # ones (S symmetric, values bit-identical). computed blocks:
                # (ib, jb) with ib <= 2*(jb//2)+1, EXCEPT the four
                # below-diagonal halves (2e+1, 2e) which HALF_GROUPS skip —
                # those are transposed too. Sources cast by superphase A
                # interleave with the phase-B matmul stream ((7,6) first:
                # the Tk(6,.) units in phase B read it); B-cast-sourced ones
                # run after phase B, just ahead of their mid consumers.
                pairs = [(ib, jb) for jb in range(n_dblk)
                         for ib in range(2 * (jb // 2) + 2, n_dblk)]
                b_sourced = [(ib, jb) for (ib, jb) in pairs if ib < 4 and jb < 2]
                a_sourced = [p for p in pairs if p not in b_sourced]
                a_sourced.sort(key=lambda p: (p[1] < 4, p[1] < 2))
                a_sourced.insert(0, (n_dblk - 1, n_dblk - 2))
                b_sourced = b_sourced + [(ib - 1, ib - 2) for ib in
                                         range(n_dblk - 2, 1, -2)]

                def transpose_group(grp, tp_pool):
                    ptp = tp_pool.tile([P, len(grp) * P], BF16, name="tps",
                                       tag="tps")
                    for k, (ibp, jb) in enumerate(grp):
                        nc.tensor.transpose(out=ptp[:, ts(k, P)],
                                            in_=sb_tiles[jb][:, ts(ibp, P)],
                                            identity=ident_sb)
                    for k, (ibp, jb) in enumerate(grp):
                        nc.vector.tensor_copy(out=sb_tiles[ibp][:, ts(jb, P)],
                                              in_=ptp[:, ts(k, P)])

                # bank layout: groups 2b, 2b+1 share bank b; only the very
                # first matmul of each bank has start=True, the partner group
                # lands on pending-zero bytes (start_tensor_calc zeroes the
                # whole 2KB bank)
                chunk_seq = [(phase, c) for phase in range(2)
                             for c in range(n_chunks)]
                xn_cache = {}

                def ensure(idx):
                    if idx < len(chunk_seq) and idx not in xn_cache:
                        xn_cache[idx] = dma_xn(chunk_seq[idx][1])

                ensure(0)
                ps_list = None
                with tc.tile_pool(name="tp_ps", bufs=2, space="PSUM") as tp_ps:
                    for idx, (phase, c) in enumerate(chunk_seq):
                        ensure(idx + 1)
                        ensure(idx + 2)
                        ensure(idx + 3)
                        groups = (spa, spb)[phase]
                        n_grp = len(groups)
                        if c == 0:
                            # phase B maps to banks 2-5: banks 0,1 hold the
                            # conv groups whose A-casts land last, so B's
                            # start=True writes never wait on them (they host
                            # the interleaved Tk units instead)
                            boff = 2 if phase == 1 else 0
                            ps_list = [s_ps.tile([P, 2 * ES], F32,
                                                 name=f"sps{b}",
                                                 tag=f"sps{b + boff}")
                                       for b in range((n_grp + 1) // 2)]
                        if idx == 1:
                            nc.sync.dma_start(out=ident_sb, in_=ident[:, :])
                        if 5 <= idx <= 12:
                            # wk/wq spread thin (one tile per hwdge queue per
                            # set) through the late-A/B stream: wk complete by
                            # idx 8, just before the Tk(6,7) groups in phase
                            # B; wq by idx 12, well before the Uq/G consumers
                            wi = 2 * (idx - 5)
                            for k in (wi, wi + 1):
                                wname, wdram = (("wk", wk), ("wq", wq))[k // n_dblk]
                                db = k % n_dblk
                                eng = nc.sync if k % 2 == 0 else nc.scalar
                                eng.dma_start(out=w_tiles[wname][db],
                                              in_=wdram[ts(db, P), :])

                        def ps_slice(g, ps_list=ps_list):
                            return ps_list[g // 2][:, (g % 2) * ES:(g % 2) * ES + ES]

                        xns, xn8s = xn_cache.pop(idx)
                        last = c == n_chunks - 1
                        # last chunk runs group-outer so each accumulator
                        # stops early and its cast drains under the rest;
                        # fp8 groups go last so their operand casts (DVE/ACT)
                        # get a head start on the matmul stream
                        gord = sorted(range(n_grp),
                                      key=lambda g: groups[g] in S_FP8_GROUPS)
                        order = ([(s, g) for g in gord for s in range(n_sub)]
                                 if last else
                                 [(s, g) for s in range(n_sub) for g in gord])
                        for s, g in order:
                            e, ib = groups[g]
                            conv = (e, ib) in S_FP8_GROUPS
                            co = P if (e, ib) in HALF_GROUPS else 0
                            wd = ES - co
                            pslc = ps_slice(g)[:, co:co + wd]
                            if conv:
                                # fp8 DoubleRow: one matmul per subtile PAIR
                                if s % 2 == 1:
                                    continue
                                nc.tensor.matmul(
                                    pslc,
                                    lhsT=xn8s[s // 2][:, :, ts(ib, P)],
                                    rhs=xn8s[s // 2][:, :,
                                                     ES * e + co:ES * e + co + wd],
                                    perf_mode=DR,
                                    start=(c == 0 and s == 0 and g % 2 == 0),
                                    stop=(last and s == n_sub - 2))
                                group_done = last and s == n_sub - 2
                            else:
                                nc.tensor.matmul(
                                    pslc, lhsT=xns[s][:, ts(ib, P)],
                                    rhs=xns[s][:, ES * e + co:ES * e + co + wd],
                                    start=(c == 0 and s == 0 and g % 2 == 0),
                                    stop=(last and s == n_sub - 1))
                                group_done = last and s == n_sub - 1
                            if group_done:
                                # alternate engines: halves the serial casts;
                                # fp8 groups divide out the SX^2 operand scale
                                scl = 1.0 / (SX * SX) if conv else 1.0
                                dst = sb_tiles[ib][:, ES * e + co:ES * e + co + wd]
                                if g % 2 == 0:
                                    nc.vector.tensor_scalar_mul(
                                        out=dst, in0=pslc, scalar1=scl)
                                else:
                                    nc.scalar.activation(
                                        out=dst, in_=pslc, func=AF.Copy,
                                        scale=scl)
                        if phase == 1 and 1 <= c <= (len(a_sourced) + 3) // 4:
                            transpose_group(a_sourced[4 * (c - 1):4 * c], tp_ps)
                        if phase == 1 and 2 <= c <= 5:
                            # Tk columns 6,7 need only superphase-A S blocks
                            # (no transposes), so they fill the DMA-paced
                            # phase-B stream; banks sps4/sps5 are idle here
                            tk_ib, tk_e = 6 + (c - 2) // 2, (c - 2) % 2
                            pt = s_ps.tile([P, 2 * ES], F32, name="tks",
                                           tag=f"sps{tk_e}")
                            tk_unit(tk_ib, tk_e, pt, first=(c <= 3),
                                    tmp_pool=post_pool)
                    transpose_group(b_sourced, tp_ps)

            # ---------------- pass-2 SBUF pools (open early for prefetch) ------
            nq = post_pool.tile([1, d], F32, name="nq", tag="nq")
            rk_col = post_pool.tile([P, n_dblk], F32, name="rk_col", tag="rk_col")
            rq_bc = post_pool.tile([P, d], F32, name="rq_bc", tag="rq_bc")
            rk625 = post_pool.tile([P, n_dblk], F32, name="rk625", tag="rk625")
            rk375 = post_pool.tile([P, n_dblk], F32, name="rk375", tag="rk375")

            with tc.tile_pool(name="xt2", bufs=2) as xt2_pool, \
                 tc.tile_pool(name="vt", bufs=2) as vt_pool, \
                 tc.tile_pool(name="gate", bufs=2) as gate_pool, \
                 tc.tile_pool(name="wtmp", bufs=3) as wtmp_pool, \
                 tc.tile_pool(name="osb", bufs=3) as osb_pool:

                def dma_xt(c):
                    tiles = []
                    for db in range(n_dblk):
                        t = xt2_pool.tile([P, NC], BF16, name=f"x2_{db}",
                                          tag=f"x2_{db}")
                        eng = nc.sync if db % 2 == 0 else nc.gpsimd
                        eng.dma_start(out=t, in_=xt[ts(db, P), ts(c, NC)])
                        tiles.append(t)
                    t8s = []
                    for j in range(n_pair):
                        t8 = xt2_pool.tile([P, 2, NC], FP8, name=f"x8_{j}",
                                           tag=f"x8_{j}")
                        for i in range(2):
                            nc.gpsimd.dma_start(
                                out=t8[:, i, :],
                                in_=xt8[2 * j * P + i * P:2 * j * P + (i + 1) * P,
                                        ts(c, NC)])
                        t8s.append(t8)
                    return tiles, t8s

                xts_cache = {0: dma_xt(0), 1: dma_xt(1)}

                def issue_vrvi_pool(c, xts_pair, ps_pool):
                    xts, x8s = xts_pair
                    vts = []
                    for eb in range(n_dblk):
                        pvr = ps_pool.tile([P, NC], F32, name="pvr", tag="midps")
                        pvi = ps_pool.tile([P, NC], F32, name="pvi", tag="midps")
                        for db in range(n_dblk):
                            nc.tensor.matmul(pvr,
                                             lhsT=w_tiles["wvr"][db][:, ts(eb, P)],
                                             rhs=xts[db], start=(db == 0),
                                             stop=(db == n_dblk - 1))
                        for j in range(n_pair):
                            nc.tensor.matmul(pvi,
                                             lhsT=wvi_tiles[j][:, :, ts(eb, P)],
                                             rhs=x8s[j], perf_mode=DR,
                                             start=(j == 0),
                                             stop=(j == n_pair - 1))
                        # gate = c0 + c1*tanh(a1 x + b1) + c2*tanh(a2 x + b2);
                        # pvi holds VI*SW*SX so fold 1/(SW*SX) into the scales
                        t1 = gate_pool.tile([P, NC], BF16, name="t1", tag="t1")
                        nc.scalar.activation(out=t1, in_=pvi, func=AF.Tanh,
                                             scale=GA1 / VI_SCL, bias=gb1)
                        t2 = gate_pool.tile([P, NC], BF16, name="t2", tag="t2")
                        nc.scalar.activation(out=t2, in_=pvi, func=AF.Tanh,
                                             scale=GA2 / VI_SCL, bias=gb2)
                        g2 = gate_pool.tile([P, NC], BF16, name="g2", tag="g2")
                        nc.vector.scalar_tensor_tensor(out=g2, in0=t2,
                                                       scalar=GC2 / GC1,
                                                       in1=t1, op0=ALU.mult,
                                                       op1=ALU.add)
                        g3 = gate_pool.tile([P, NC], BF16, name="g3", tag="g3")
                        nc.vector.tensor_scalar(out=g3, in0=g2, scalar1=GC1,
                                                scalar2=GC0, op0=ALU.mult,
                                                op1=ALU.add)
                        vt = vt_pool.tile([P, NC], BF16, name=f"vt{eb}",
                                          tag=f"vt{eb}")
                        nc.vector.tensor_mul(out=vt, in0=g3, in1=pvr)
                        vts.append(vt)
                    return vts

                # wvi8/wq8 stream in under the Tk matmuls; first needed by
                # VRVI(0) / Uq at the end of the mid phase (wvr spreads
                # across the G loop below)
                for j in range(n_pair):
                    for i in range(2):
                        nc.sync.dma_start(
                            out=wvi_tiles[j][:, i, :],
                            in_=wvi8[2 * j * P + i * P:2 * j * P + (i + 1) * P, :])
                        nc.scalar.dma_start(
                            out=wq8_tiles[j][:, i, :],
                            in_=wq8[2 * j * P + i * P:2 * j * P + (i + 1) * P, :])

                # ---------------- Mid: Tk, Uq, G, norms, A' ----------------
                with tc.tile_pool(name="mid_ps", bufs=5, space="PSUM") as mid_ps, \
                     tc.tile_pool(name="nrm_ps", bufs=1, space="PSUM") as nrm_ps:

                    # Tk = S @ WkT ; nk2 = colsum(WkT * Tk) in transpose-
                    # readiness order. The W*T products accumulate on the DVE
                    # in bf16 (partition sums commute across ib blocks), so
                    # the partition-reduce is one matmul per 128-block,
                    # emitted directly in COLUMN layout.
                    nkc_ps = nrm_ps.tile([P, n_dblk], F32, name="nkc", tag="nkc")
                    nq2_ps = {}
                    # the S->fp8 pair casts for Uq ride these iterations
                    for ib_pos, ib in enumerate([4, 5, 2, 3, 0, 1][:n_dblk]):
                        for e in range(n_ef):
                            pt = mid_ps.tile([P, ef], F32, name="pt", tag="midps")
                            tk_unit(ib, e, pt, first=False,
                                    tmp_pool=wtmp_pool)
                            # S -> fp8 pair casts (scaled 2^-5), one per (ib,e)
                            ci = ib_pos * n_ef + e
                            if 0 <= ci < 2 * n_pair:
                                j, i = divmod(ci, 2)
                                if ci % 2 == 0:
                                    nc.vector.tensor_scalar_mul(
                                        out=s8_tiles[j][:, i, :],
                                        in0=sb_tiles[2 * j + i], scalar1=SXS)
                                else:
                                    nc.scalar.activation(
                                        out=s8_tiles[j][:, i, :],
                                        in_=sb_tiles[2 * j + i], func=AF.Copy,
                                        scale=SXS)

                    # Uq = S @ WqT in fp8 DoubleRow (Uq feeds only the norm
                    # nq2 = colsum(WqT * Uq) — a sum of squares, so the ~4%
                    # fp8 noise averages to ~0.1% on rq); psum carries
                    # Uq * UQ_SCL, divided out in the rsqrt activation
                    for e in range(n_ef):
                        for ib in range(n_dblk):
                            pt = mid_ps.tile([P, ef], F32, name="pu", tag="midps")
                            for j in range(n_pair):
                                nc.tensor.matmul(pt,
                                                 lhsT=s8_tiles[j][:, :, ts(ib, P)],
                                                 rhs=wq8_tiles[j][:, :, ts(e, ef)],
                                                 perf_mode=DR, start=(j == 0),
                                                 stop=(j == n_pair - 1))
                            tmp = wtmp_pool.tile([P, ef], BF16, name="tmq", tag="wtmpb")
                            nc.vector.tensor_mul(out=tmp, in0=pt,
                                                 in1=w_tiles["wq"][ib][:, ts(e, ef)])
                            if ib == 0:
                                nc.vector.tensor_copy(out=acc_q[e], in_=tmp)
                            else:
                                nc.vector.tensor_add(out=acc_q[e], in0=acc_q[e], in1=tmp)
                            if e == 0 and ib == 1:
                                # nk2 partition-reduce straight into column
                                # layout [P, n_dblk]: one tiny matmul per
                                # 128-block; single bank, one start=True
                                for db in range(n_dblk):
                                    ee, dd = divmod(db, n_dblk // n_ef)
                                    nc.tensor.matmul(
                                        nkc_ps[:, db:db + 1],
                                        lhsT=acc_k[ee][:, ts(dd, P)],
                                        rhs=onesb, start=(db == 0), stop=True)
                            if e == 0 and ib == 3:
                                # rk = 1/sqrt(nk2) on ACT, column form (the
                                # +1e-5 in the reference is a 1.6e-7 relative
                                # effect at these norms)
                                nc.scalar.activation(out=rk_col, in_=nkc_ps,
                                                     func=AF.Abs_reciprocal_sqrt)
                            if e == 0 and ib == 4:
                                nc.vector.tensor_scalar_mul(out=rk625, in0=rk_col,
                                                            scalar1=0.625)
                                nc.vector.tensor_scalar_mul(out=rk375, in0=rk_col,
                                                            scalar1=0.375)
                    # G = Tk^T @ WqT ; A' = rk * (0.625 G + 0.375 |G|) from PSUM.
                    # The trailing nq2 ones-matmul, the rq chain and the first
                    # VRVI chunk all interleave with the G groups so the PE
                    # never idles across the mid->pass2 transition.
                    g_groups = [(eb, e) for eb in range(n_dblk) for e in range(n_ef)]
                    for gi, (eb, e) in enumerate(g_groups):
                        if gi % 2 == 0 and gi // 2 < n_dblk:
                            # wvr trickles in across the G phase; first needed
                            # by VRVI(0) right after it
                            db = gi // 2
                            eng = nc.sync if db % 2 == 0 else nc.scalar
                            eng.dma_start(out=w_tiles["wvr"][db],
                                          in_=wvr[ts(db, P), :])
                        pg = mid_ps.tile([P, ef], F32, name="pg", tag="midps")
                        for db in range(n_dblk):
                            nc.tensor.matmul(pg, lhsT=tkb_tiles[db][:, ts(eb, P)],
                                             rhs=w_tiles["wq"][db][:, ts(e, ef)],
                                             start=(db == 0), stop=(db == n_dblk - 1))
                        if gi == 0:
                            for ee in range(n_ef):
                                nq2_ps[ee] = nrm_ps.tile([1, ef], F32,
                                                         name=f"nq2{ee}",
                                                         tag=f"nrm{ee}")
                                nc.tensor.matmul(nq2_ps[ee], lhsT=onesb,
                                                 rhs=acc_q[ee],
                                                 start=True, stop=True)
                        if gi == 2:
                            # rq = 1/sqrt(nq2) on ACT under the G matmuls;
                            # scale removes the fp8 operand scaling (UQ_SCL)
                            for ee in range(n_ef):
                                nc.scalar.activation(out=nq[0:1, ts(ee, ef)],
                                                     in_=nq2_ps[ee],
                                                     func=AF.Abs_reciprocal_sqrt,
                                                     scale=1.0 / UQ_SCL)
                        if gi == 5:
                            for ee in range(n_ef):
                                pb = mid_ps.tile([P, ef], F32, name="pb", tag="midps")
                                nc.tensor.matmul(pb, lhsT=ones_row,
                                                 rhs=nq[0:1, ts(ee, ef)],
                                                 start=True, stop=True)
                                nc.vector.tensor_copy(out=rq_bc[:, ts(ee, ef)], in_=pb)
                        tabs = wtmp_pool.tile([P, ef], F32, name="tabs", tag="wtmp")
                        nc.scalar.activation(out=tabs, in_=pg, func=AF.Abs,
                                             scale=rk375[:, eb:eb + 1])
                        nc.vector.scalar_tensor_tensor(
                            out=a_tiles[eb][:, ts(e, ef)], in0=pg,
                            scalar=rk625[:, eb:eb + 1], in1=tabs,
                            op0=ALU.mult, op1=ALU.add)

                    def issue_out(c, vts, ps_pool):
                        for s in range(n_sub):
                            for e in range(n_ef):
                                po = ps_pool.tile([P, ef], F32, name="po", tag="midps")
                                for eb in range(n_dblk):
                                    nc.tensor.matmul(po, lhsT=vts[eb][:, ts(s, P)],
                                                     rhs=a_tiles[eb][:, ts(e, ef)],
                                                     start=(eb == 0),
                                                     stop=(eb == n_dblk - 1))
                                ot = osb_pool.tile([P, ef], BF16, name="ot", tag="osb")
                                nc.vector.tensor_mul(out=ot, in0=po,
                                                     in1=rq_bc[:, ts(e, ef)])
                                nc.sync.dma_start(
                                    out=out_d[c * NC + s * P:c * NC + (s + 1) * P,
                                              ts(e, ef)],
                                    in_=ot)

                    # first two VRVI chunks AND the first out phase run out of
                    # the mid psum pool (same tile shape): every later pool-
                    # boundary bank reuse is then separated from its previous
                    # consumer by a full VRVI or out phase of PE work, so the
                    # transition never waits on a trailing gate chain
                    vts_fifo = [issue_vrvi_pool(0, xts_cache.pop(0), mid_ps),
                                issue_vrvi_pool(1, xts_cache.pop(1), mid_ps)]
                    xts_cache[2] = dma_xt(2)
                    issue_out(0, vts_fifo[0], mid_ps)

                # ---------------- Pass 2: VRVI / out pipeline ----------------
                with tc.tile_pool(name="vrvi_ps", bufs=4, space="PSUM") as vrvi_ps, \
                     tc.tile_pool(name="out_ps", bufs=4, space="PSUM") as out_ps:

                    for c in range(1, n_chunks):
                        if c + 1 < n_chunks:
                            vts_fifo.append(
                                issue_vrvi_pool(c + 1, xts_cache.pop(c + 1), vrvi_ps))
                        if c + 2 < n_chunks:
                            xts_cache[c + 2] = dma_xt(c + 2)
                        issue_out(c, vts_fifo[c], out_ps)
    nc.compile()
    return nc


_PROGRAM_CACHE = {}


def _get_program(n, d):
    key = (n, d)
    if key not in _PROGRAM_CACHE:
        _PROGRAM_CACHE[key] = build_program(n, d)
    return _PROGRAM_CACHE[key]


def _numpy_reference(x, Wvr, bvr, Wvi, bvi, Wk, bk, Wq, bq):
    """Slow fp32 fallback (never expected to run: biases are zeros)."""
    out = np.empty_like(x)
    for b in range(x.shape[0]):
        xb = x[b].astype(np.float64)
        vr = xb @ Wvr.T.astype(np.float64) + bvr
        vi = xb @ Wvi.T.astype(np.float64) + bvi
        v = vr * np.tanh(np.logaddexp(0.0, vi))
        k = xb @ Wk.T.astype(np.float64) + bk
        q = xb @ Wq.T.astype(np.float64) + bq
        kn = k / (np.linalg.norm(k, axis=0, keepdims=True) + 1e-5)
        qn = q / (np.linalg.norm(q, axis=0, keepdims=True) + 1e-5)
        g = kn.T @ qn
        a = 0.625 * g + 0.375 * np.abs(g)
        out[b] = (v @ a).astype(np.float32)
    return out


def kernel(_run_kwargs=None, **inputs):
    run_kwargs = _run_kwargs or {}
    x = np.asarray(inputs["x"], dtype=np.float32)
    Wvr = np.asarray(inputs["Wvr"], dtype=np.float32)
    Wvi = np.asarray(inputs["Wvi"], dtype=np.float32)
    Wk = np.asarray(inputs["Wk"], dtype=np.float32)
    Wq = np.asarray(inputs["Wq"], dtype=np.float32)
    bvr, bvi = np.asarray(inputs["bvr"]), np.asarray(inputs["bvi"])
    bk, bq = np.asarray(inputs["bk"]), np.asarray(inputs["bq"])

    if any(np.any(b != 0) for b in (bvr, bvi, bk, bq)):
        return _numpy_reference(x, Wvr, bvr, Wvi, bvi, Wk, bk, Wq, bq)

    b, n, d = x.shape
    assert b == B and n == N_FULL and d == D_FULL, (b, n, d)

    bf16 = ml_dtypes.bfloat16
    fp8 = ml_dtypes.float8_e4m3
    wvr_t = np.ascontiguousarray(Wvr.T).astype(bf16)
    wk_t = np.ascontiguousarray(Wk.T).astype(bf16)
    wq_t = np.ascontiguousarray(Wq.T).astype(bf16)
    wvi8 = np.clip(np.ascontiguousarray(Wvi.T) * SW, -240.0, 240.0).astype(fp8)
    wq8_h = np.clip(np.ascontiguousarray(Wq.T) * SW, -240.0, 240.0).astype(fp8)

    ident = np.eye(P, dtype=bf16)
    in_maps = []
    for i in range(N_CORES):
        xti = np.ascontiguousarray(x[i].T)
        in_maps.append({
            "xn": x[i].astype(bf16),
            "xt": xti.astype(bf16),
            "xt8": np.clip(xti * SX, -240.0, 240.0).astype(fp8),
            "wvr": wvr_t, "wvi8": wvi8, "wk": wk_t, "wq": wq_t,
            "wq8": wq8_h, "ident": ident,
        })

    nc = _get_program(n, d)
    from concourse.bass_utils import run_bass_kernel_spmd
    res = run_bass_kernel_spmd(nc, in_maps, core_ids=list(range(N_CORES)), **run_kwargs)
    out = np.stack([np.asarray(res.results[i]["out"]).astype(np.float32)
                    for i in range(N_CORES)], axis=0)
    if run_kwargs:
        kernel.last_results = res
    return out


# revision 66
# speedup vs baseline: 1.1726x; 1.1726x over previous
"""Trainium2 Bass kernel for nn_GatedFeedForward (gated feed-forward with
feature attention).

Reference computation per batch b (B=8, N=4096, D=1024):
    VR = x @ Wvr.T ; VI = x @ Wvi.T            (biases are zero)
    V  = VR * tanh(softplus(VI))
    K  = x @ Wk.T  ; Q  = x @ Wq.T
    Kn = K / (||K||_col + 1e-5) ; Qn = Q / (||Q||_col + 1e-5)   (norm over N)
    A  = smu(Kn.T @ Qn)     # == leaky-relu slope 0.25 == 0.625x + 0.375|x|
    out = V @ A
Sharding: pure data-parallel over batch — one batch per NeuronCore.

Key algebraic restructure: with S = X^T X (D x D, one N-contraction),
    K^T Q        = WkT^T S WqT          (WkT = Wk.T, [in,out])
    ||K_d||^2    = colsum(WkT * (S WkT))
    ||Q_e||^2    = colsum(WqT * (S WqT))
so the K/Q path costs ~0.56 N*D^2 (S upper triangle) plus three D^3
matmuls instead of 3 N*D^2. leaky's positive homogeneity folds rk into
A's rows and rq into the output tiles.

Precision: bf16 matmuls with fp32 PSUM, EXCEPT fp8-e4m3 DoubleRow
(2x PE rate, both operands fp8, 256-deep contraction per instruction)
for three noise-tolerant contractions (tolerance 2e-2, measured
1.89e-2, all verified against a numpy e4m3 simulation first):
  - VI (gate input): only passes through the saturating gate
    tanh(softplus(.)), so ~5% fp8 noise -> ~1.5e-2 output noise.
  - Uq = S WqT: feeds only nq2 = colsum(WqT*Uq), a sum of squares
    where fp8 noise averages to ~0.1% on rq.
  - ~31% of S's blocks (6 of 20 groups + mirrors): each fraction f of
    S in fp8 adds sqrt(f)*1.7e-2 output noise.
Operand pow2 scales (W*4096, x*32, S*2^-5) are folded into the gate
activation constants / rsqrt scale / PSUM->bf16 cast scales.

The gate tanh(softplus(x)) is evaluated as c0 + c1*tanh(a1 x + b1)
+ c2*tanh(a2 x + b2) (max abs err 3.9e-3): both ops hit the resident
tanh activation table — no table switches, no slow DVE reciprocal.

Pass-1 S uses 256-wide column spans (triangle = 20 of 32 blocks
instead of 12 of 16 at 512-wide). Two [P,256] accumulation groups
share each 2 KB PSUM bank; since start_tensor_calc zeroes the WHOLE
bank, only the temporally-first matmul of each bank carries start=True
and the partner group's first matmul lands on pending-zero bytes.

Schedule per core (pass 1 is right at the DMA roofline — the xn
stream crosses HBM twice — so scheduling there is DMA-first):
  Pass 1: S in two column superphases (cols 512:1024 = 12 groups/6
      banks, then cols 0:512 = 8 groups/4 banks at +2 bank offset so
      phase B never waits on phase A's last casts), accumulated in
      PSUM across all 8 sequence chunks; cast to bf16 (scaled for the
      fp8 groups) at superphase end. The lower-triangle transpose
      reconstruction and the Tk columns 6,7 (which depend only on
      phase-A blocks) interleave into phase B's DMA-paced stream;
      fp8 groups run last within each chunk so their operand casts
      (DVE/ACT) stay ahead of the PE. wk/wq trickle one tile per
      queue per chunk-set; wvr trickles through the mid G loop.
  Mid:    Tk (+nk2 via per-block ones-matmuls straight into column
      layout), Uq in fp8 (+nq2), G -> A' from PSUM; S->fp8 pair casts
      ride the Tk iterations.
  Pass 2: VRVI(c) / out(c) software-pipelined with lag 1 so the PE
      never waits on the gate chain; xt/xt8 split across sync/gpsimd
      queues, prefetched 2 chunks ahead. Output stored bf16, upcast
      on host.
"""

import numpy as np
import ml_dtypes

import concourse.bass as bass
import concourse.tile as tile
from concourse import bacc, mybir
from concourse.bass import ts

F32 = mybir.dt.float32
BF16 = mybir.dt.bfloat16
FP8 = mybir.dt.float8e4
AF = mybir.ActivationFunctionType
ALU = mybir.AluOpType
DR = mybir.MatmulPerfMode.DoubleRow

B = 8
N_FULL = 4096
D_FULL = 1024
N_CORES = 8

P = 128  # SBUF partitions
NC = 512  # sequence chunk
EF = 512  # free-dim span per mid/out matmul / psum bank
ES = 256  # pass-1 S column span (two spans share a psum bank)

# fp8 operand scales (pow2; folded into gate constants)
SW = 4096.0  # Wvi/Wq scale: xavier max 0.0542*4096 = 222 < 240
SX = 32.0    # x scale: |x|max ~5.2*32 = 167 < 240
VI_SCL = SW * SX
SXS = 2.0 ** -5  # S scale: diag ~N=4096 -> 128 < 240
UQ_SCL = SW * SXS  # Uq psum carries Uq * 128

# tanh-sum fit of tanh(softplus(x)), max |err| 3.9e-3 on [-12, 12]
GC0, GC1, GA1, GB1 = 0.50022747, 0.32785149, 0.8261997, -0.02962021
GC2, GA2, GB2 = 0.17216236, 0.57575332, 0.75023909


def build_program(n=N_FULL, d=D_FULL):
    """Build the single-core SPMD Bass program for one [n, d] batch."""
    assert n % NC == 0 and d % P == 0
    n_chunks = n // NC
    n_sub = NC // P  # 128-row subtiles per chunk
    n_dblk = d // P  # feature blocks
    ef = min(EF, d)
    n_ef = d // ef
    n_es = d // ES  # pass-1 column spans
    n_pair = n_dblk // 2  # fp8 DoubleRow pair-blocks

    nc = bacc.Bacc("TRN2", target_bir_lowering=False, debug=False,
                   num_devices=N_CORES)
    ident = nc.dram_tensor("ident", [P, P], BF16, kind="ExternalInput")
    xn = nc.dram_tensor("xn", [n, d], BF16, kind="ExternalInput")
    xt = nc.dram_tensor("xt", [d, n], BF16, kind="ExternalInput")
    xt8 = nc.dram_tensor("xt8", [d, n], FP8, kind="ExternalInput")
    wvr = nc.dram_tensor("wvr", [d, d], BF16, kind="ExternalInput")
    wvi8 = nc.dram_tensor("wvi8", [d, d], FP8, kind="ExternalInput")
    wq8 = nc.dram_tensor("wq8", [d, d], FP8, kind="ExternalInput")
    wk = nc.dram_tensor("wk", [d, d], BF16, kind="ExternalInput")
    wq = nc.dram_tensor("wq", [d, d], BF16, kind="ExternalInput")
    out_d = nc.dram_tensor("out", [n, d], BF16, kind="ExternalOutput")

    with tile.TileContext(nc) as tc:
        with tc.tile_pool(name="const", bufs=1) as const_pool, \
             tc.tile_pool(name="w", bufs=1) as w_pool, \
             tc.tile_pool(name="sb", bufs=1) as sb_pool, \
             tc.tile_pool(name="tkb", bufs=1) as tkb_pool, \
             tc.tile_pool(name="post", bufs=1) as post_pool, \
             tc.tile_pool(name="apost", bufs=1) as ap_pool:
            # bf16 ones for the norm partition-reduce matmuls: fp32 moving
            # operands stream at half rate and add PE dtype-mode switches
            onesb = const_pool.tile([P, 1], BF16, name="onesb", tag="onesb")
            nc.vector.memset(onesb, 1.0)
            ones_row = const_pool.tile([1, P], F32, name="ones_row", tag="onesr")
            nc.vector.memset(ones_row, 1.0)
            gb1 = const_pool.tile([P, 1], F32, name="gb1", tag="gb1")
            nc.vector.memset(gb1, GB1)
            gb2 = const_pool.tile([P, 1], F32, name="gb2", tag="gb2")
            nc.vector.memset(gb2, GB2)
            ident_sb = const_pool.tile([P, P], BF16, name="ident", tag="ident")

            w_tiles = {}
            for wname in ("wk", "wq", "wvr"):
                w_tiles[wname] = [
                    w_pool.tile([P, d], BF16, name=f"{wname}{db}", tag=f"{wname}{db}")
                    for db in range(n_dblk)
                ]
            # Wvi/Wq as fp8 pair-tiles: [:, i, :] holds W rows 256j+128i..+127
            wvi_tiles = [w_pool.tile([P, 2, d], FP8, name=f"wvi{j}", tag=f"wvi{j}")
                         for j in range(n_pair)]
            wq8_tiles = [w_pool.tile([P, 2, d], FP8, name=f"wq8{j}", tag=f"wq8{j}")
                         for j in range(n_pair)]
            # S as fp8 pair-tiles (cast from sb_tiles during Tk) for the Uq
            # DoubleRow matmuls
            s8_tiles = [w_pool.tile([P, 2, d], FP8, name=f"s8{j}", tag=f"s8{j}")
                        for j in range(n_pair)]

            sb_tiles = [sb_pool.tile([P, d], BF16, name=f"s{ib}", tag=f"s{ib}")
                        for ib in range(n_dblk)]
            tkb_tiles = [tkb_pool.tile([P, d], BF16, name=f"tk{ib}", tag=f"tk{ib}")
                         for ib in range(n_dblk)]
            a_tiles = [ap_pool.tile([P, d], BF16, name=f"a{db}", tag=f"a{db}")
                       for db in range(n_dblk)]
            acc_k = {e: post_pool.tile([P, ef], BF16, name=f"acck{e}",
                                       tag=f"acck{e}") for e in range(n_ef)}
            acc_q = {e: post_pool.tile([P, ef], BF16, name=f"accq{e}",
                                       tag=f"accq{e}") for e in range(n_ef)}

            def tk_unit(ib, e, pt, first, tmp_pool):
                """One Tk output block: Tk[:, ib-block x e-span] = S @ WkT,
                bf16 cast to tkb_tiles, and the nk2 partial product
                acc_k[e] (+)= WkT * Tk."""
                for jb in range(n_dblk):
                    nc.tensor.matmul(pt, lhsT=sb_tiles[jb][:, ts(ib, P)],
                                     rhs=w_tiles["wk"][jb][:, ts(e, ef)],
                                     start=(jb == 0), stop=(jb == n_dblk - 1))
                # cast on ACT (idle here) to keep the DVE under the matmul
                # pace with the accumulate added
                nc.scalar.activation(out=tkb_tiles[ib][:, ts(e, ef)],
                                     in_=pt, func=AF.Copy)
                tmp = tmp_pool.tile([P, ef], BF16, name="tmp", tag="wtmpb")
                nc.vector.tensor_mul(out=tmp, in0=pt,
                                     in1=w_tiles["wk"][ib][:, ts(e, ef)])
                if first:
                    nc.vector.tensor_copy(out=acc_k[e], in_=tmp)
                else:
                    nc.vector.tensor_add(out=acc_k[e], in0=acc_k[e], in1=tmp)

            # ---------------- Pass 1: S = X^T X, PSUM-resident ----------------
            # Upper triangle at [128-row x 256-col] granularity:
            #   col-span e (256 wide) needs row-blocks ib with 128*ib <= 256e+255
            # Two superphases, split 12/8 groups (6/4 banks) so two banks stay
            # free for the transpose scratch that interleaves with phase B.
            tri = [(e, ib) for e in range(n_es - 1, -1, -1)
                   for ib in range(min(n_dblk, (ES * (e + 1)) // P))]
            spa, spb = tri[:12], tri[12:]
            # ~31% of S (incl. mirrored blocks) in fp8 DoubleRow: adds
            # ~1e-2 quadrature noise (sim: 1.64e-2 -> 1.88e-2, limit 2e-2)
            # and halves those groups' PE time
            S_FP8_GROUPS = {(3, 0), (3, 1), (3, 2), (3, 3), (0, 0), (0, 1)}
            # groups (e, 2e+1) straddle the diagonal: compute only their
            # upper 128-col half, the lower half comes from a transpose
            HALF_GROUPS = {(e, 2 * e + 1) for e in range(n_es)}

            with tc.tile_pool(name="xn1", bufs=4) as xn_pool, \
                 tc.tile_pool(name="xn8", bufs=2) as xn8_pool, \
                 tc.tile_pool(name="s_ps", bufs=1, space="PSUM") as s_ps:

                # at kernel start the serial per-queue transfer stream paces
                # the PE: both hwdge queues carry xn throughout pass 1, and
                # the first chunk-set is halved across them per subtile
                dma_state = {"set": 0}

                # superphase B touches only columns < 768 (rhs spans e<=2,
                # lhsT blocks ib<=5), so its re-stream loads 3/4 width
                wb = ES * (max(e for e, ib in spb) + 1)

                def dma_xn(c):
                    iset = dma_state["set"]
                    dma_state["set"] += 1
                    w = d if iset < n_chunks else wb
                    tiles = []
                    for s in range(n_sub):
                        t = xn_pool.tile([P, d], BF16, name=f"xn{s}", tag=f"xn{s}")
                        if iset < 2:
                            nc.sync.dma_start(
                                out=t[:, 0:w // 2],
                                in_=xn[c * NC + s * P:c * NC + (s + 1) * P,
                                       0:w // 2])
                            nc.scalar.dma_start(
                                out=t[:, w // 2:w],
                                in_=xn[c * NC + s * P:c * NC + (s + 1) * P,
                                       w // 2:w])
                        else:
                            eng = nc.sync if (s + iset) % 2 == 0 else nc.scalar
                            eng.dma_start(
                                out=t[:, 0:w],
                                in_=xn[c * NC + s * P:c * NC + (s + 1) * P, 0:w])
                        tiles.append(t)
                    # fp8 pair casts (x * SX) for the DoubleRow S groups; the
                    # psum then carries S * SX^2, divided out at the cast
                    p8s = []
                    for sp in range(n_sub // 2):
                        t8 = xn8_pool.tile([P, 2, d], FP8, name=f"xn8_{sp}",
                                           tag=f"xn8_{sp}")
                        for i in range(2):
                            if (sp + i) % 2 == 0:
                                nc.vector.tensor_scalar_mul(
                                    out=t8[:, i, 0:w], in0=tiles[2 * sp + i][:, 0:w],
                                    scalar1=SX)
                            else:
                                nc.scalar.activation(
                                    out=t8[:, i, 0:w], in_=tiles[2 * sp + i][:, 0:w],
                                    func=AF.Copy, scale=SX)
                        p8s.append(t8)
                    return tiles, p8s

                # lower-left S blocks = transposes of the computed upper-right
                # ones (S symmetric, values bit-identical). computed blocks:
                # (ib, jb) with ib <= 2*(jb//2)+1, EXCEPT the four
                # below-diagonal halves (2e+1, 2e) which HALF_GROUPS skip —
                # those are transposed too. Sources cast by superphase A
                # interleave with the phase-B matmul stream ((7,6) first:
                # the Tk(6,.) units in phase B read it); B-cast-sourced ones
                # run after phase B, just ahead of their mid consumers.
                pairs = [(ib, jb) for jb in range(n_dblk)
                         for ib in range(2 * (jb // 2) + 2, n_dblk)]
                b_sourced = [(ib, jb) for (ib, jb) in pairs if ib < 4 and jb < 2]
                a_sourced = [p for p in pairs if p not in b_sourced]
                a_sourced.sort(key=lambda p: (p[1] < 4, p[1] < 2))
                a_sourced.insert(0, (n_dblk - 1, n_dblk - 2))
                b_sourced = b_sourced + [(ib - 1, ib - 2) for ib in
                                         range(n_dblk - 2, 1, -2)]

                def transpose_group(grp, tp_pool):
                    ptp = tp_pool.tile([P, len(grp) * P], BF16, name="tps",
                                       tag="tps")
                    for k, (ibp, jb) in enumerate(grp):
                        nc.tensor.transpose(out=ptp[:, ts(k, P)],
                                            in_=sb_tiles[jb][:, ts(ibp, P)],
                                            identity=ident_sb)
                    for k, (ibp, jb) in enumerate(grp):
                        nc.vector.tensor_copy(out=sb_tiles[ibp][:, ts(jb, P)],
                                              in_=ptp[:, ts(k, P)])

                # bank layout: groups 2b, 2b+1 share bank b; only the very
                # first matmul of each bank has start=True, the partner group
                # lands on pending-zero bytes (start_tensor_calc zeroes the
                # whole 2KB bank)
                chunk_seq = [(phase, c) for phase in range(2)
                             for c in range(n_chunks)]
                xn_cache = {}

                def ensure(idx):
                    if idx < len(chunk_seq) and idx not in xn_cache:
                        xn_cache[idx] = dma_xn(chunk_seq[idx][1])

                ensure(0)
                ps_list = None
                with tc.tile_pool(name="tp_ps", bufs=2, space="PSUM") as tp_ps:
                    for idx, (phase, c) in enumerate(chunk_seq):
                        ensure(idx + 1)
                        ensure(idx + 2)
                        ensure(idx + 3)
                        groups = (spa, spb)[phase]
                        n_grp = len(groups)
                        if c == 0:
                            # phase B maps to banks 2-5: banks 0,1 hold the
                            # conv groups whose A-casts land last, so B's
                            # start=True writes never wait on them (they host
                            # the interleaved Tk units instead)
                            boff = 2 if phase == 1 else 0
                            ps_list = [s_ps.tile([P, 2 * ES], F32,
                                                 name=f"sps{b}",
                                                 tag=f"sps{b + boff}")
                                       for b in range((n_grp + 1) // 2)]
                        if idx == 1:
                            nc.sync.dma_start(out=ident_sb, in_=ident[:, :])
                        if 5 <= idx <= 12:
                            # wk/wq spread thin (one tile per hwdge queue per
                            # set) through the late-A/B stream: wk complete by
                            # idx 8, just before the Tk(6,7) groups in phase
                            # B; wq by idx 12, well before the Uq/G consumers
                            wi = 2 * (idx - 5)
                            for k in (wi, wi + 1):
                                wname, wdram = (("wk", wk), ("wq", wq))[k // n_dblk]
                                db = k % n_dblk
                                eng = nc.sync if k % 2 == 0 else nc.scalar
                                eng.dma_start(out=w_tiles[wname][db],
                                              in_=wdram[ts(db, P), :])

                        def ps_slice(g, ps_list=ps_list):
                            return ps_list[g // 2][:, (g % 2) * ES:(g % 2) * ES + ES]

                        xns, xn8s = xn_cache.pop(idx)
                        last = c == n_chunks - 1
                        # last chunk runs group-outer so each accumulator
                        # stops early and its cast drains under the rest;
                        # fp8 groups go last so their operand casts (DVE/ACT)
                        # get a head start on the matmul stream
                        gord = sorted(range(n_grp),
                                      key=lambda g: groups[g] in S_FP8_GROUPS)
                        order = ([(s, g) for g in gord for s in range(n_sub)]
                                 if last else
                                 [(s, g) for s in range(n_sub) for g in gord])
                        for s, g in order:
                            e, ib = groups[g]
                            conv = (e, ib) in S_FP8_GROUPS
                            co = P if (e, ib) in HALF_GROUPS else 0
                            wd = ES - co
                            pslc = ps_slice(g)[:, co:co + wd]
                            if conv:
                                # fp8 DoubleRow: one matmul per subtile PAIR
                                if s % 2 == 1:
                                    continue
                                nc.tensor.matmul(
                                    pslc,
                                    lhsT=xn8s[s // 2][:, :, ts(ib, P)],
                                    rhs=xn8s[s // 2][:, :,
                                                     ES * e + co:ES * e + co + wd],
                                    perf_mode=DR,
                                    start=(c == 0 and s == 0 and g % 2 == 0),
                                    stop=(last and s == n_sub - 2))
                                group_done = last and s == n_sub - 2
                            else:
                                nc.tensor.matmul(
                                    pslc, lhsT=xns[s][:, ts(ib, P)],
                                    rhs=xns[s][:, ES * e + co:ES * e + co + wd],
                                    start=(c == 0 and s == 0 and g % 2 == 0),
                                    stop=(last and s == n_sub - 1))
                                group_done = last and s == n_sub - 1
                            if group_done:
                                # alternate engines: halves the serial casts;
                                # fp8 groups divide out the SX^2 operand scale
                                scl = 1.0 / (SX * SX) if conv else 1.0
                                dst = sb_tiles[ib][:, ES * e + co:ES * e + co + wd]
                                if g % 2 == 0:
                                    nc.vector.tensor_scalar_mul(
                                        out=dst, in0=pslc, scalar1=scl)
                                else:
                                    nc.scalar.activation(
                                        out=dst, in_=pslc, func=AF.Copy,
                                        scale=scl)
                        if phase == 1 and 1 <= c <= (len(a_sourced) + 3) // 4:
                            transpose_group(a_sourced[4 * (c - 1):4 * c], tp_ps)
                        if phase == 1 and 2 <= c <= 5:
                            # Tk columns 6,7 need only superphase-A S blocks
                            # (no transposes), so they fill the DMA-paced
                            # phase-B stream; banks sps4/sps5 are idle here
                            tk_ib, tk_e = 6 + (c - 2) // 2, (c - 2) % 2
                            pt = s_ps.tile([P, 2 * ES], F32, name="tks",
                                           tag=f"sps{tk_e}")
                            tk_unit(tk_ib, tk_e, pt, first=(c <= 3),
                                    tmp_pool=post_pool)
                    transpose_group(b_sourced, tp_ps)

            # ---------------- pass-2 SBUF pools (open early for prefetch) ------
            nq = post_pool.tile([1, d], F32, name="nq", tag="nq")
            rk_col = post_pool.tile([P, n_dblk], F32, name="rk_col", tag="rk_col")
            rq_bc = post_pool.tile([P, d], F32, name="rq_bc", tag="rq_bc")
            rk625 = post_pool.tile([P, n_dblk], F32, name="rk625", tag="rk625")
            rk375 = post_pool.tile([P, n_dblk], F32, name="rk375", tag="rk375")

            with tc.tile_pool(name="xt2", bufs=2) as xt2_pool, \
                 tc.tile_pool(name="vt", bufs=2) as vt_pool, \
                 tc.tile_pool(name="gate", bufs=2) as gate_pool, \
                 tc.tile_pool(name="wtmp", bufs=3) as wtmp_pool, \
                 tc.tile_pool(name="osb", bufs=3) as osb_pool:

                def dma_xt(c):
                    tiles = []
                    for db in range(n_dblk):
                        t = xt2_pool.tile([P, NC], BF16, name=f"x2_{db}",
                                          tag=f"x2_{db}")
                        eng = nc.sync if db % 2 == 0 else nc.gpsimd
                        eng.dma_start(out=t, in_=xt[ts(db, P), ts(c, NC)])
                        tiles.append(t)
                    t8s = []
                    for j in range(n_pair):
                        t8 = xt2_pool.tile([P, 2, NC], FP8, name=f"x8_{j}",
                                           tag=f"x8_{j}")
                        for i in range(2):
                            nc.gpsimd.dma_start(
                                out=t8[:, i, :],
                                in_=xt8[2 * j * P + i * P:2 * j * P + (i + 1) * P,
                                        ts(c, NC)])
                        t8s.append(t8)
                    return tiles, t8s

                xts_cache = {0: dma_xt(0), 1: dma_xt(1)}

                def issue_vrvi_pool(c, xts_pair, ps_pool):
                    xts, x8s = xts_pair
                    vts = []
                    for eb in range(n_dblk):
                        pvr = ps_pool.tile([P, NC], F32, name="pvr", tag="midps")
                        pvi = ps_pool.tile([P, NC], F32, name="pvi", tag="midps")
                        for db in range(n_dblk):
                            nc.tensor.matmul(pvr,
                                             lhsT=w_tiles["wvr"][db][:, ts(eb, P)],
                                             rhs=xts[db], start=(db == 0),
                                             stop=(db == n_dblk - 1))
                        for j in range(n_pair):
                            nc.tensor.matmul(pvi,
                                             lhsT=wvi_tiles[j][:, :, ts(eb, P)],
                                             rhs=x8s[j], perf_mode=DR,
                                             start=(j == 0),
                                             stop=(j == n_pair - 1))
                        # gate = c0 + c1*tanh(a1 x + b1) + c2*tanh(a2 x + b2);
                        # pvi holds VI*SW*SX so fold 1/(SW*SX) into the scales
                        t1 = gate_pool.tile([P, NC], BF16, name="t1", tag="t1")
                        nc.scalar.activation(out=t1, in_=pvi, func=AF.Tanh,
                                             scale=GA1 / VI_SCL, bias=gb1)
                        t2 = gate_pool.tile([P, NC], BF16, name="t2", tag="t2")
                        nc.scalar.activation(out=t2, in_=pvi, func=AF.Tanh,
                                             scale=GA2 / VI_SCL, bias=gb2)
                        g2 = gate_pool.tile([P, NC], BF16, name="g2", tag="g2")
                        nc.vector.scalar_tensor_tensor(out=g2, in0=t2,
                                                       scalar=GC2 / GC1,
                                                       in1=t1, op0=ALU.mult,
                                                       op1=ALU.add)
                        g3 = gate_pool.tile([P, NC], BF16, name="g3", tag="g3")
                        nc.vector.tensor_scalar(out=g3, in0=g2, scalar1=GC1,
                                                scalar2=GC0, op0=ALU.mult,
                                                op1=ALU.add)
                        vt = vt_pool.tile([P, NC], BF16, name=f"vt{eb}",
                                          tag=f"vt{eb}")
                        nc.vector.tensor_mul(out=vt, in0=g3, in1=pvr)
                        vts.append(vt)
                    return vts

                # wvi8/wq8 stream in under the Tk matmuls; first needed by
                # VRVI(0) / Uq at the end of the mid phase (wvr spreads
                # across the G loop below)
                for j in range(n_pair):
                    for i in range(2):
                        nc.sync.dma_start(
                            out=wvi_tiles[j][:, i, :],
                            in_=wvi8[2 * j * P + i * P:2 * j * P + (i + 1) * P, :])
                        nc.scalar.dma_start(
                            out=wq8_tiles[j][:, i, :],
                            in_=wq8[2 * j * P + i * P:2 * j * P + (i + 1) * P, :])

                # ---------------- Mid: Tk, Uq, G, norms, A' ----------------
                with tc.tile_pool(name="mid_ps", bufs=5, space="PSUM") as mid_ps, \
                     tc.tile_pool(name="nrm_ps", bufs=1, space="PSUM") as nrm_ps:

                    # Tk = S @ WkT ; nk2 = colsum(WkT * Tk) in transpose-
                    # readiness order. The W*T products accumulate on the DVE
                    # in bf16 (partition sums commute across ib blocks), so
                    # the partition-reduce is one matmul per 128-block,
                    # emitted directly in COLUMN layout.
                    nkc_ps = nrm_ps.tile([P, n_dblk], F32, name="nkc", tag="nkc")
                    nq2_ps = {}
                    # the S->fp8 pair casts for Uq ride these iterations
                    for ib_pos, ib in enumerate([4, 5, 2, 3, 0, 1][:n_dblk]):
                        for e in range(n_ef):
                            pt = mid_ps.tile([P, ef], F32, name="pt", tag="midps")
                            tk_unit(ib, e, pt, first=False,
                                    tmp_pool=wtmp_pool)
                            # S -> fp8 pair casts (scaled 2^-5), one per (ib,e)
                            ci = ib_pos * n_ef + e
                            if 0 <= ci < 2 * n_pair:
                                j, i = divmod(ci, 2)
                                if ci % 2 == 0:
                                    nc.vector.tensor_scalar_mul(
                                        out=s8_tiles[j][:, i, :],
                                        in0=sb_tiles[2 * j + i], scalar1=SXS)
                                else:
                                    nc.scalar.activation(
                                        out=s8_tiles[j][:, i, :],
                                        in_=sb_tiles[2 * j + i], func=AF.Copy,
                                        scale=SXS)

                    # Uq = S @ WqT in fp8 DoubleRow (Uq feeds only the norm
                    # nq2 = colsum(WqT * Uq) — a sum of squares, so the ~4%
                    # fp8 noise averages to ~0.1% on rq); psum carries
                    # Uq * UQ_SCL, divided out in the rsqrt activation
                    for e in range(n_ef):
                        for ib in range(n_dblk):
                            pt = mid_ps.tile([P, ef], F32, name="pu", tag="midps")
                            for j in range(n_pair):
                                nc.tensor.matmul(pt,
                                                 lhsT=s8_tiles[j][:, :, ts(ib, P)],
                                                 rhs=wq8_tiles[j][:, :, ts(e, ef)],
                                                 perf_mode=DR, start=(j == 0),
                                                 stop=(j == n_pair - 1))
                            tmp = wtmp_pool.tile([P, ef], BF16, name="tmq", tag="wtmpb")
                            nc.vector.tensor_mul(out=tmp, in0=pt,
                                                 in1=w_tiles["wq"][ib][:, ts(e, ef)])
                            if ib == 0:
                                nc.vector.tensor_copy(out=acc_q[e], in_=tmp)
                            else:
                                nc.vector.tensor_add(out=acc_q[e], in0=acc_q[e], in1=tmp)
                            if e == 0 and ib == 1:
                                # nk2 partition-reduce straight into column
                                # layout [P, n_dblk]: one tiny matmul per
                                # 128-block; single bank, one start=True
                                for db in range(n_dblk):
                                    ee, dd = divmod(db, n_dblk // n_ef)
                                    nc.tensor.matmul(
                                        nkc_ps[:, db:db + 1],
                                        lhsT=acc_k[ee][:, ts(dd, P)],
                                        rhs=onesb, start=(db == 0), stop=True)
                            if e == 0 and ib == 3:
                                # rk = 1/sqrt(nk2) on ACT, column form (the
                                # +1e-5 in the reference is a 1.6e-7 relative
                                # effect at these norms)
                                nc.scalar.activation(out=rk_col, in_=nkc_ps,
                                                     func=AF.Abs_reciprocal_sqrt)
                            if e == 0 and ib == 4:
                                nc.vector.tensor_scalar_mul(out=rk625, in0=rk_col,
                                                            scalar1=0.625)
                                nc.vector.tensor_scalar_mul(out=rk375, in0=rk_col,
                                                            scalar1=0.375)
                    # G = Tk^T @ WqT ; A' = rk * (0.625 G + 0.375 |G|) from PSUM.
                    # The trailing nq2 ones-matmul, the rq chain and the first
                    # VRVI chunk all interleave with the G groups so the PE
                    # never idles across the mid->pass2 transition.
                    g_groups = [(eb, e) for eb in range(n_dblk) for e in range(n_ef)]
                    for gi, (eb, e) in enumerate(g_groups):
                        if gi % 2 == 0 and gi // 2 < n_dblk:
                            # wvr trickles in across the G phase; first needed
                            # by VRVI(0) right after it
                            db = gi // 2
                            eng = nc.sync if db % 2 == 0 else nc.scalar
                            eng.dma_start(out=w_tiles["wvr"][db],
                                          in_=wvr[ts(db, P), :])
                        pg = mid_ps.tile([P, ef], F32, name="pg", tag="midps")
                        for db in range(n_dblk):
                            nc.tensor.matmul(pg, lhsT=tkb_tiles[db][:, ts(eb, P)],
                                             rhs=w_tiles["wq"][db][:, ts(e, ef)],
                                             start=(db == 0), stop=(db == n_dblk - 1))
                        if gi == 0:
                            for ee in range(n_ef):
                                nq2_ps[ee] = nrm_ps.tile([1, ef], F32,
                                                         name=f"nq2{ee}",
                                                         tag=f"nrm{ee}")
                                nc.tensor.matmul(nq2_ps[ee], lhsT=onesb,
                                                 rhs=acc_q[ee],
                                                 start=True, stop=True)
                        if gi == 2:
                            # rq = 1/sqrt(nq2) on ACT under the G matmuls;
                            # scale removes the fp8 operand scaling (UQ_SCL)
                            for ee in range(n_ef):
                                nc.scalar.activation(out=nq[0:1, ts(ee, ef)],
                                                     in_=nq2_ps[ee],
                                                     func=AF.Abs_reciprocal_sqrt,
                                                     scale=1.0 / UQ_SCL)
                        if gi == 5:
                            for ee in range(n_ef):
                                pb = mid_ps.tile([P, ef], F32, name="pb", tag="midps")
                                nc.tensor.matmul(pb, lhsT=ones_row,
                                                 rhs=nq[0:1, ts(ee, ef)],
                                                 start=True, stop=True)
                                nc.vector.tensor_copy(out=rq_bc[:, ts(ee, ef)], in_=pb)
                        tabs = wtmp_pool.tile([P, ef], F32, name="tabs", tag="wtmp")
                        nc.scalar.activation(out=tabs, in_=pg, func=AF.Abs,
                                             scale=rk375[:, eb:eb + 1])
                        nc.vector.scalar_tensor_tensor(
                            out=a_tiles[eb][:, ts(e, ef)], in0=pg,
                            scalar=rk625[:, eb:eb + 1], in1=tabs,
                            op0=ALU.mult, op1=ALU.add)

                    def issue_out(c, vts, ps_pool):
                        for s in range(n_sub):
                            for e in range(n_ef):
                                po = ps_pool.tile([P, ef], F32, name="po", tag="midps")
                                for eb in range(n_dblk):
                                    nc.tensor.matmul(po, lhsT=vts[eb][:, ts(s, P)],
                                                     rhs=a_tiles[eb][:, ts(e, ef)],
                                                     start=(eb == 0),
                                                     stop=(eb == n_dblk - 1))
                                ot = osb_pool.tile([P, ef], BF16, name="ot", tag="osb")
                                nc.vector.tensor_mul(out=ot, in0=po,
                                                     in1=rq_bc[:, ts(e, ef)])
                                nc.sync.dma_start(
                                    out=out_d[c * NC + s * P:c * NC + (s + 1) * P,
                                              ts(e, ef)],
                                    in_=ot)

                    # first two VRVI chunks AND the first out phase run out of
                    # the mid psum pool (same tile shape): every later pool-
                    # boundary bank reuse is then separated from its previous
                    # consumer by a full VRVI or out phase of PE work, so the
                    # transition never waits on a trailing gate chain
                    vts_fifo = [issue_vrvi_pool(0, xts_cache.pop(0), mid_ps),
                                issue_vrvi_pool(1, xts_cache.pop(1), mid_ps)]
                    xts_cache[2] = dma_xt(2)
                    issue_out(0, vts_fifo[0], mid_ps)

                # ---------------- Pass 2: VRVI / out pipeline ----------------
                with tc.tile_pool(name="vrvi_ps", bufs=4, space="PSUM") as vrvi_ps, \
                     tc.tile_pool(name="out_ps", bufs=4, space="PSUM") as out_ps:

                    for c in range(1, n_chunks):
                        if c + 1 < n_chunks:
                            vts_fifo.append(
                                issue_vrvi_pool(c + 1, xts_cache.pop(c + 1), vrvi_ps))
                        if c + 2 < n_chunks:
                            xts_cache[c + 2] = dma_xt(c + 2)
                        issue_out(c, vts_fifo[c], out_ps)
    nc.compile()
    return nc


_PROGRAM_CACHE = {}


def _get_program(n, d):
    key = (n, d)
    if key not in _PROGRAM_CACHE:
        _PROGRAM_CACHE[key] = build_program(n, d)
    return _PROGRAM_CACHE[key]


def _numpy_reference(x, Wvr, bvr, Wvi, bvi, Wk, bk, Wq, bq):
    """Slow fp32 fallback (never expected to run: biases are zeros)."""
    out = np.empty_like(x)
    for b in range(x.shape[0]):
        xb = x[b].astype(np.float64)
        vr = xb @ Wvr.T.astype(np.float64) + bvr
        vi = xb @ Wvi.T.astype(np.float64) + bvi
        v = vr * np.tanh(np.logaddexp(0.0, vi))
        k = xb @ Wk.T.astype(np.float64) + bk
        q = xb @ Wq.T.astype(np.float64) + bq
        kn = k / (np.linalg.norm(k, axis=0, keepdims=True) + 1e-5)
        qn = q / (np.linalg.norm(q, axis=0, keepdims=True) + 1e-5)
        g = kn.T @ qn
        a = 0.625 * g + 0.375 * np.abs(g)
        out[b] = (v @ a).astype(np.float32)
    return out


def kernel(_run_kwargs=None, **inputs):
    run_kwargs = _run_kwargs or {}
    x = np.asarray(inputs["x"], dtype=np.float32)
    Wvr = np.asarray(inputs["Wvr"], dtype=np.float32)
    Wvi = np.asarray(inputs["Wvi"], dtype=np.float32)
    Wk = np.asarray(inputs["Wk"], dtype=np.float32)
    Wq = np.asarray(inputs["Wq"], dtype=np.float32)
    bvr, bvi = np.asarray(inputs["bvr"]), np.asarray(inputs["bvi"])
    bk, bq = np.asarray(inputs["bk"]), np.asarray(inputs["bq"])

    if any(np.any(b != 0) for b in (bvr, bvi, bk, bq)):
        return _numpy_reference(x, Wvr, bvr, Wvi, bvi, Wk, bk, Wq, bq)

    b, n, d = x.shape
    assert b == B and n == N_FULL and d == D_FULL, (b, n, d)

    bf16 = ml_dtypes.bfloat16
    fp8 = ml_dtypes.float8_e4m3
    wvr_t = np.ascontiguousarray(Wvr.T).astype(bf16)
    wk_t = np.ascontiguousarray(Wk.T).astype(bf16)
    wq_t = np.ascontiguousarray(Wq.T).astype(bf16)
    wvi8 = np.clip(np.ascontiguousarray(Wvi.T) * SW, -240.0, 240.0).astype(fp8)
    wq8_h = np.clip(np.ascontiguousarray(Wq.T) * SW, -240.0, 240.0).astype(fp8)

    ident = np.eye(P, dtype=bf16)
    in_maps = []
    for i in range(N_CORES):
        xti = np.ascontiguousarray(x[i].T)
        in_maps.append({
            "xn": x[i].astype(bf16),
            "xt": xti.astype(bf16),
            "xt8": np.clip(xti * SX, -240.0, 240.0).astype(fp8),
            "wvr": wvr_t, "wvi8": wvi8, "wk": wk_t, "wq": wq_t,
            "wq8": wq8_h, "ident": ident,
        })

    nc = _get_program(n, d)
    from concourse.bass_utils import run_bass_kernel_spmd
    res = run_bass_kernel_spmd(nc, in_maps, core_ids=list(range(N_CORES)), **run_kwargs)
    out = np.stack([np.asarray(res.results[i]["out"]).astype(np.float32)
                    for i in range(N_CORES)], axis=0)
    if run_kwargs:
        kernel.last_results = res
    return out


# revision 68
# speedup vs baseline: 1.1852x; 1.0108x over previous
"""Trainium2 Bass kernel for nn_GatedFeedForward (gated feed-forward with
feature attention).

Reference computation per batch b (B=8, N=4096, D=1024):
    VR = x @ Wvr.T ; VI = x @ Wvi.T            (biases are zero)
    V  = VR * tanh(softplus(VI))
    K  = x @ Wk.T  ; Q  = x @ Wq.T
    Kn = K / (||K||_col + 1e-5) ; Qn = Q / (||Q||_col + 1e-5)   (norm over N)
    A  = smu(Kn.T @ Qn)     # == leaky-relu slope 0.25 == 0.625x + 0.375|x|
    out = V @ A
Sharding: pure data-parallel over batch — one batch per NeuronCore.

Key algebraic restructure: with S = X^T X (D x D, one N-contraction),
    K^T Q        = WkT^T S WqT          (WkT = Wk.T, [in,out])
    ||K_d||^2    = colsum(WkT * (S WkT))
    ||Q_e||^2    = colsum(WqT * (S WqT))
so the K/Q path costs ~0.56 N*D^2 (S upper triangle) plus three D^3
matmuls instead of 3 N*D^2. leaky's positive homogeneity folds rk into
A's rows and rq into the output tiles.

Precision: bf16 matmuls with fp32 PSUM, EXCEPT fp8-e4m3 DoubleRow
(2x PE rate, both operands fp8, 256-deep contraction per instruction)
for three noise-tolerant contractions (tolerance 2e-2, measured
1.89e-2, all verified against a numpy e4m3 simulation first):
  - VI (gate input): only passes through the saturating gate
    tanh(softplus(.)), so ~5% fp8 noise -> ~1.5e-2 output noise.
  - Uq = S WqT: feeds only nq2 = colsum(WqT*Uq), a sum of squares
    where fp8 noise averages to ~0.1% on rq.
  - ~31% of S's blocks (6 of 20 groups + mirrors): each fraction f of
    S in fp8 adds sqrt(f)*1.7e-2 output noise.
Operand pow2 scales (W*4096, x*32, S*2^-5) are folded into the gate
activation constants / rsqrt scale / PSUM->bf16 cast scales.

The gate tanh(softplus(x)) is evaluated as c0 + c1*tanh(a1 x + b1)
+ c2*tanh(a2 x + b2) (max abs err 3.9e-3): both ops hit the resident
tanh activation table — no table switches, no slow DVE reciprocal.

Pass-1 S uses 256-wide column spans (triangle = 20 of 32 blocks
instead of 12 of 16 at 512-wide). Two [P,256] accumulation groups
share each 2 KB PSUM bank; since start_tensor_calc zeroes the WHOLE
bank, only the temporally-first matmul of each bank carries start=True
and the partner group's first matmul lands on pending-zero bytes.

Schedule per core (pass 1 is right at the DMA roofline — the xn
stream crosses HBM twice — so scheduling there is DMA-first):
  Pass 1: S in two column superphases (cols 512:1024 = 12 groups/6
      banks, then cols 0:512 = 8 groups/4 banks at +2 bank offset so
      phase B never waits on phase A's last casts), accumulated in
      PSUM across all 8 sequence chunks; cast to bf16 (scaled for the
      fp8 groups) at superphase end. The lower-triangle transpose
      reconstruction and the Tk columns 6,7 (which depend only on
      phase-A blocks) interleave into phase B's DMA-paced stream;
      fp8 groups run last within each chunk so their operand casts
      (DVE/ACT) stay ahead of the PE. wk/wq trickle one tile per
      queue per chunk-set; wvr trickles through the mid G loop.
  Mid:    Tk (+nk2 via per-block ones-matmuls straight into column
      layout), Uq in fp8 (+nq2), G -> A' from PSUM; S->fp8 pair casts
      ride the Tk iterations.
  Pass 2: VRVI(c) / out(c) software-pipelined with lag 1 so the PE
      never waits on the gate chain; xt/xt8 split across sync/gpsimd
      queues, prefetched 2 chunks ahead. Output stored bf16, upcast
      on host.
"""

import numpy as np
import ml_dtypes

import concourse.bass as bass
import concourse.tile as tile
from concourse import bacc, mybir
from concourse.bass import ts

F32 = mybir.dt.float32
BF16 = mybir.dt.bfloat16
FP8 = mybir.dt.float8e4
AF = mybir.ActivationFunctionType
ALU = mybir.AluOpType
DR = mybir.MatmulPerfMode.DoubleRow

B = 8
N_FULL = 4096
D_FULL = 1024
N_CORES = 8

P = 128  # SBUF partitions
NC = 512  # sequence chunk
EF = 512  # free-dim span per mid/out matmul / psum bank
ES = 256  # pass-1 S column span (two spans share a psum bank)

# fp8 operand scales (pow2; folded into gate constants)
SW = 4096.0  # Wvi/Wq scale: xavier max 0.0542*4096 = 222 < 240
SX = 32.0    # x scale: |x|max ~5.2*32 = 167 < 240
VI_SCL = SW * SX
SXS = 2.0 ** -5  # S scale: diag ~N=4096 -> 128 < 240
UQ_SCL = SW * SXS  # Uq psum carries Uq * 128

# tanh-sum fit of tanh(softplus(x)), max |err| 3.9e-3 on [-12, 12]
GC0, GC1, GA1, GB1 = 0.50022747, 0.32785149, 0.8261997, -0.02962021
GC2, GA2, GB2 = 0.17216236, 0.57575332, 0.75023909


def build_program(n=N_FULL, d=D_FULL):
    """Build the single-core SPMD Bass program for one [n, d] batch."""
    assert n % NC == 0 and d % P == 0
    n_chunks = n // NC
    n_sub = NC // P  # 128-row subtiles per chunk
    n_dblk = d // P  # feature blocks
    ef = min(EF, d)
    n_ef = d // ef
    n_es = d // ES  # pass-1 column spans
    n_pair = n_dblk // 2  # fp8 DoubleRow pair-blocks

    nc = bacc.Bacc("TRN2", target_bir_lowering=False, debug=False,
                   num_devices=N_CORES)
    ident = nc.dram_tensor("ident", [P, P], BF16, kind="ExternalInput")
    xn = nc.dram_tensor("xn", [n, d], BF16, kind="ExternalInput")
    xt = nc.dram_tensor("xt", [d, n], BF16, kind="ExternalInput")
    xt8 = nc.dram_tensor("xt8", [d, n], FP8, kind="ExternalInput")
    wvr = nc.dram_tensor("wvr", [d, d], BF16, kind="ExternalInput")
    wvi8 = nc.dram_tensor("wvi8", [d, d], FP8, kind="ExternalInput")
    wq8 = nc.dram_tensor("wq8", [d, d], FP8, kind="ExternalInput")
    wk = nc.dram_tensor("wk", [d, d], BF16, kind="ExternalInput")
    wq = nc.dram_tensor("wq", [d, d], BF16, kind="ExternalInput")
    out_d = nc.dram_tensor("out", [n, d], BF16, kind="ExternalOutput")

    with tile.TileContext(nc) as tc:
        with tc.tile_pool(name="const", bufs=1) as const_pool, \
             tc.tile_pool(name="w", bufs=1) as w_pool, \
             tc.tile_pool(name="sb", bufs=1) as sb_pool, \
             tc.tile_pool(name="tkb", bufs=1) as tkb_pool, \
             tc.tile_pool(name="post", bufs=1) as post_pool, \
             tc.tile_pool(name="apost", bufs=1) as ap_pool:
            # bf16 ones for the norm partition-reduce matmuls: fp32 moving
            # operands stream at half rate and add PE dtype-mode switches
            onesb = const_pool.tile([P, 1], BF16, name="onesb", tag="onesb")
            nc.vector.memset(onesb, 1.0)
            ones_row = const_pool.tile([1, P], F32, name="ones_row", tag="onesr")
            nc.vector.memset(ones_row, 1.0)
            gb1 = const_pool.tile([P, 1], F32, name="gb1", tag="gb1")
            nc.vector.memset(gb1, GB1)
            gb2 = const_pool.tile([P, 1], F32, name="gb2", tag="gb2")
            nc.vector.memset(gb2, GB2)
            ident_sb = const_pool.tile([P, P], BF16, name="ident", tag="ident")

            w_tiles = {}
            for wname in ("wk", "wq", "wvr"):
                w_tiles[wname] = [
                    w_pool.tile([P, d], BF16, name=f"{wname}{db}", tag=f"{wname}{db}")
                    for db in range(n_dblk)
                ]
            # Wvi/Wq as fp8 pair-tiles: [:, i, :] holds W rows 256j+128i..+127
            wvi_tiles = [w_pool.tile([P, 2, d], FP8, name=f"wvi{j}", tag=f"wvi{j}")
                         for j in range(n_pair)]
            wq8_tiles = [w_pool.tile([P, 2, d], FP8, name=f"wq8{j}", tag=f"wq8{j}")
                         for j in range(n_pair)]
            # S as fp8 pair-tiles (cast from sb_tiles during Tk) for the Uq
            # DoubleRow matmuls
            s8_tiles = [w_pool.tile([P, 2, d], FP8, name=f"s8{j}", tag=f"s8{j}")
                        for j in range(n_pair)]

            sb_tiles = [sb_pool.tile([P, d], BF16, name=f"s{ib}", tag=f"s{ib}")
                        for ib in range(n_dblk)]
            tkb_tiles = [tkb_pool.tile([P, d], BF16, name=f"tk{ib}", tag=f"tk{ib}")
                         for ib in range(n_dblk)]
            a_tiles = [ap_pool.tile([P, d], BF16, name=f"a{db}", tag=f"a{db}")
                       for db in range(n_dblk)]
            acc_k = {e: post_pool.tile([P, ef], BF16, name=f"acck{e}",
                                       tag=f"acck{e}") for e in range(n_ef)}
            acc_q = {e: post_pool.tile([P, ef], BF16, name=f"accq{e}",
                                       tag=f"accq{e}") for e in range(n_ef)}

            def tk_unit(ib, e, pt, first, tmp_pool):
                """One Tk output block: Tk[:, ib-block x e-span] = S @ WkT,
                bf16 cast to tkb_tiles, and the nk2 partial product
                acc_k[e] (+)= WkT * Tk."""
                for jb in range(n_dblk):
                    nc.tensor.matmul(pt, lhsT=sb_tiles[jb][:, ts(ib, P)],
                                     rhs=w_tiles["wk"][jb][:, ts(e, ef)],
                                     start=(jb == 0), stop=(jb == n_dblk - 1))
                # cast on ACT (idle here) to keep the DVE under the matmul
                # pace with the accumulate added
                nc.scalar.activation(out=tkb_tiles[ib][:, ts(e, ef)],
                                     in_=pt, func=AF.Copy)
                tmp = tmp_pool.tile([P, ef], BF16, name="tmp", tag="wtmpb")
                nc.vector.tensor_mul(out=tmp, in0=pt,
                                     in1=w_tiles["wk"][ib][:, ts(e, ef)])
                if first:
                    nc.vector.tensor_copy(out=acc_k[e], in_=tmp)
                else:
                    nc.vector.tensor_add(out=acc_k[e], in0=acc_k[e], in1=tmp)

            # ---------------- Pass 1: S = X^T X, PSUM-resident ----------------
            # Upper triangle at [128-row x 256-col] granularity:
            #   col-span e (256 wide) needs row-blocks ib with 128*ib <= 256e+255
            # Two superphases, split 12/8 groups (6/4 banks) so two banks stay
            # free for the transpose scratch that interleaves with phase B.
            tri = [(e, ib) for e in range(n_es - 1, -1, -1)
                   for ib in range(min(n_dblk, (ES * (e + 1)) // P))]
            spa, spb = tri[:12], tri[12:]
            # ~31% of S (incl. mirrored blocks) in fp8 DoubleRow: adds
            # ~1e-2 quadrature noise (sim: 1.64e-2 -> 1.88e-2, limit 2e-2)
            # and halves those groups' PE time
            S_FP8_GROUPS = {(3, 0), (3, 1), (3, 2), (3, 3), (0, 0), (0, 1)}
            # groups (e, 2e+1) straddle the diagonal: compute only their
            # upper 128-col half, the lower half comes from a transpose
            HALF_GROUPS = {(e, 2 * e + 1) for e in range(n_es)}

            with tc.tile_pool(name="xn1", bufs=4) as xn_pool, \
                 tc.tile_pool(name="xn8", bufs=2) as xn8_pool, \
                 tc.tile_pool(name="s_ps", bufs=1, space="PSUM") as s_ps:

                # at kernel start the serial per-queue transfer stream paces
                # the PE: both hwdge queues carry xn throughout pass 1, and
                # the first chunk-set is halved across them per subtile
                dma_state = {"set": 0}

                # superphase B touches only columns < 768 (rhs spans e<=2,
                # lhsT blocks ib<=5), so its re-stream loads 3/4 width
                wb = ES * (max(e for e, ib in spb) + 1)

                def dma_xn(c):
                    iset = dma_state["set"]
                    dma_state["set"] += 1
                    w = d if iset < n_chunks else wb
                    tiles = []
                    for s in range(n_sub):
                        t = xn_pool.tile([P, d], BF16, name=f"xn{s}", tag=f"xn{s}")
                        if iset < 2:
                            nc.sync.dma_start(
                                out=t[:, 0:w // 2],
                                in_=xn[c * NC + s * P:c * NC + (s + 1) * P,
                                       0:w // 2])
                            nc.scalar.dma_start(
                                out=t[:, w // 2:w],
                                in_=xn[c * NC + s * P:c * NC + (s + 1) * P,
                                       w // 2:w])
                        else:
                            eng = nc.sync if (s + iset) % 2 == 0 else nc.scalar
                            eng.dma_start(
                                out=t[:, 0:w],
                                in_=xn[c * NC + s * P:c * NC + (s + 1) * P, 0:w])
                        tiles.append(t)
                    # fp8 pair casts (x * SX) for the DoubleRow S groups; the
                    # psum then carries S * SX^2, divided out at the cast.
                    # Cast ONLY the column spans the conv groups read: phase A
                    # touches [0:d/2] (lhsT) + [d-ES:d] (rhs), phase B only
                    # [0:2*ES] — the casts gate the PE via the in-order
                    # DVE/ACT queues, so width is latency
                    p8s = []
                    for sp in range(n_sub // 2):
                        t8 = xn8_pool.tile([P, 2, d], FP8, name=f"xn8_{sp}",
                                           tag=f"xn8_{sp}")
                        for i in range(2):
                            src = tiles[2 * sp + i]
                            if iset < n_chunks:
                                nc.vector.tensor_scalar_mul(
                                    out=t8[:, i, 0:d // 2],
                                    in0=src[:, 0:d // 2], scalar1=SX)
                                nc.scalar.activation(
                                    out=t8[:, i, d - ES:d],
                                    in_=src[:, d - ES:d],
                                    func=AF.Copy, scale=SX)
                            elif (sp + i) % 2 == 0:
                                nc.vector.tensor_scalar_mul(
                                    out=t8[:, i, 0:ES],
                                    in0=src[:, 0:ES], scalar1=SX)
                            else:
                                nc.scalar.activation(
                                    out=t8[:, i, 0:ES],
                                    in_=src[:, 0:ES],
                                    func=AF.Copy, scale=SX)
                        p8s.append(t8)
                    return tiles, p8s

                # lower-left S blocks = transposes of the computed upper-right
                # ones (S symmetric, values bit-identical). computed blocks:
                # (ib, jb) with ib <= 2*(jb//2)+1, EXCEPT the four
                # below-diagonal halves (2e+1, 2e) which HALF_GROUPS skip —
                # those are transposed too. Sources cast by superphase A
                # interleave with the phase-B matmul stream ((7,6) first:
                # the Tk(6,.) units in phase B read it); B-cast-sourced ones
                # run after phase B, just ahead of their mid consumers.
                pairs = [(ib, jb) for jb in range(n_dblk)
                         for ib in range(2 * (jb // 2) + 2, n_dblk)]
                b_sourced = [(ib, jb) for (ib, jb) in pairs if ib < 4 and jb < 2]
                a_sourced = [p for p in pairs if p not in b_sourced]
                a_sourced.sort(key=lambda p: (p[1] < 4, p[1] < 2))
                a_sourced.insert(0, (n_dblk - 1, n_dblk - 2))
                b_sourced = b_sourced + [(ib - 1, ib - 2) for ib in
                                         range(n_dblk - 2, 1, -2)]

                def transpose_group(grp, tp_pool):
                    ptp = tp_pool.tile([P, len(grp) * P], BF16, name="tps",
                                       tag="tps")
                    for k, (ibp, jb) in enumerate(grp):
                        nc.tensor.transpose(out=ptp[:, ts(k, P)],
                                            in_=sb_tiles[jb][:, ts(ibp, P)],
                                            identity=ident_sb)
                    for k, (ibp, jb) in enumerate(grp):
                        nc.vector.tensor_copy(out=sb_tiles[ibp][:, ts(jb, P)],
                                              in_=ptp[:, ts(k, P)])

                # bank layout: groups 2b, 2b+1 share bank b; only the very
                # first matmul of each bank has start=True, the partner group
                # lands on pending-zero bytes (start_tensor_calc zeroes the
                # whole 2KB bank)
                chunk_seq = [(phase, c) for phase in range(2)
                             for c in range(n_chunks)]
                xn_cache = {}

                def ensure(idx):
                    if idx < len(chunk_seq) and idx not in xn_cache:
                        xn_cache[idx] = dma_xn(chunk_seq[idx][1])

                ensure(0)
                ps_list = None
                with tc.tile_pool(name="tp_ps", bufs=2, space="PSUM") as tp_ps:
                    for idx, (phase, c) in enumerate(chunk_seq):
                        ensure(idx + 1)
                        ensure(idx + 2)
                        ensure(idx + 3)
                        groups = (spa, spb)[phase]
                        n_grp = len(groups)
                        if c == 0:
                            # phase B maps to banks 2-5: banks 0,1 hold the
                            # conv groups whose A-casts land last, so B's
                            # start=True writes never wait on them (they host
                            # the interleaved Tk units instead)
                            boff = 2 if phase == 1 else 0
                            ps_list = [s_ps.tile([P, 2 * ES], F32,
                                                 name=f"sps{b}",
                                                 tag=f"sps{b + boff}")
                                       for b in range((n_grp + 1) // 2)]
                        if idx == 1:
                            nc.sync.dma_start(out=ident_sb, in_=ident[:, :])
                        if 5 <= idx <= 12:
                            # wk/wq spread thin (one tile per hwdge queue per
                            # set) through the late-A/B stream: wk complete by
                            # idx 8, just before the Tk(6,7) groups in phase
                            # B; wq by idx 12, well before the Uq/G consumers
                            wi = 2 * (idx - 5)
                            for k in (wi, wi + 1):
                                wname, wdram = (("wk", wk), ("wq", wq))[k // n_dblk]
                                db = k % n_dblk
                                eng = nc.sync if k % 2 == 0 else nc.scalar
                                eng.dma_start(out=w_tiles[wname][db],
                                              in_=wdram[ts(db, P), :])

                        def ps_slice(g, ps_list=ps_list):
                            return ps_list[g // 2][:, (g % 2) * ES:(g % 2) * ES + ES]

                        xns, xn8s = xn_cache.pop(idx)
                        last = c == n_chunks - 1
                        # last chunk runs group-outer so each accumulator
                        # stops early and its cast drains under the rest;
                        # fp8 groups go last so their operand casts (DVE/ACT)
                        # get a head start on the matmul stream
                        gord = sorted(range(n_grp),
                                      key=lambda g: groups[g] in S_FP8_GROUPS)
                        order = ([(s, g) for g in gord for s in range(n_sub)]
                                 if last else
                                 [(s, g) for s in range(n_sub) for g in gord])
                        for s, g in order:
                            e, ib = groups[g]
                            conv = (e, ib) in S_FP8_GROUPS
                            co = P if (e, ib) in HALF_GROUPS else 0
                            wd = ES - co
                            pslc = ps_slice(g)[:, co:co + wd]
                            if conv:
                                # fp8 DoubleRow: one matmul per subtile PAIR
                                if s % 2 == 1:
                                    continue
                                nc.tensor.matmul(
                                    pslc,
                                    lhsT=xn8s[s // 2][:, :, ts(ib, P)],
                                    rhs=xn8s[s // 2][:, :,
                                                     ES * e + co:ES * e + co + wd],
                                    perf_mode=DR,
                                    start=(c == 0 and s == 0 and g % 2 == 0),
                                    stop=(last and s == n_sub - 2))
                                group_done = last and s == n_sub - 2
                            else:
                                nc.tensor.matmul(
                                    pslc, lhsT=xns[s][:, ts(ib, P)],
                                    rhs=xns[s][:, ES * e + co:ES * e + co + wd],
                                    start=(c == 0 and s == 0 and g % 2 == 0),
                                    stop=(last and s == n_sub - 1))
                                group_done = last and s == n_sub - 1
                            if group_done:
                                # alternate engines: halves the serial casts;
                                # fp8 groups divide out the SX^2 operand scale
                                scl = 1.0 / (SX * SX) if conv else 1.0
                                dst = sb_tiles[ib][:, ES * e + co:ES * e + co + wd]
                                if g % 2 == 0:
                                    nc.vector.tensor_scalar_mul(
                                        out=dst, in0=pslc, scalar1=scl)
                                else:
                                    nc.scalar.activation(
                                        out=dst, in_=pslc, func=AF.Copy,
                                        scale=scl)
                        if phase == 1 and 1 <= c <= (len(a_sourced) + 3) // 4:
                            transpose_group(a_sourced[4 * (c - 1):4 * c], tp_ps)
                        if phase == 1 and 2 <= c <= 5:
                            # Tk columns 6,7 need only superphase-A S blocks
                            # (no transposes), so they fill the DMA-paced
                            # phase-B stream; banks sps4/sps5 are idle here
                            tk_ib, tk_e = 6 + (c - 2) // 2, (c - 2) % 2
                            pt = s_ps.tile([P, 2 * ES], F32, name="tks",
                                           tag=f"sps{tk_e}")
                            tk_unit(tk_ib, tk_e, pt, first=(c <= 3),
                                    tmp_pool=post_pool)
                    transpose_group(b_sourced, tp_ps)

            # ---------------- pass-2 SBUF pools (open early for prefetch) ------
            nq = post_pool.tile([1, d], F32, name="nq", tag="nq")
            rk_col = post_pool.tile([P, n_dblk], F32, name="rk_col", tag="rk_col")
            rq_bc = post_pool.tile([P, d], F32, name="rq_bc", tag="rq_bc")
            rk625 = post_pool.tile([P, n_dblk], F32, name="rk625", tag="rk625")
            rk375 = post_pool.tile([P, n_dblk], F32, name="rk375", tag="rk375")

            with tc.tile_pool(name="xt2", bufs=2) as xt2_pool, \
                 tc.tile_pool(name="vt", bufs=2) as vt_pool, \
                 tc.tile_pool(name="gate", bufs=2) as gate_pool, \
                 tc.tile_pool(name="wtmp", bufs=3) as wtmp_pool, \
                 tc.tile_pool(name="osb", bufs=3) as osb_pool:

                def dma_xt(c):
                    tiles = []
                    for db in range(n_dblk):
                        t = xt2_pool.tile([P, NC], BF16, name=f"x2_{db}",
                                          tag=f"x2_{db}")
                        eng = nc.sync if db % 2 == 0 else nc.gpsimd
                        eng.dma_start(out=t, in_=xt[ts(db, P), ts(c, NC)])
                        tiles.append(t)
                    t8s = []
                    for j in range(n_pair):
                        t8 = xt2_pool.tile([P, 2, NC], FP8, name=f"x8_{j}",
                                           tag=f"x8_{j}")
                        for i in range(2):
                            nc.gpsimd.dma_start(
                                out=t8[:, i, :],
                                in_=xt8[2 * j * P + i * P:2 * j * P + (i + 1) * P,
                                        ts(c, NC)])
                        t8s.append(t8)
                    return tiles, t8s

                xts_cache = {0: dma_xt(0), 1: dma_xt(1)}

                def issue_vrvi_pool(c, xts_pair, ps_pool):
                    xts, x8s = xts_pair
                    vts = []
                    for eb in range(n_dblk):
                        pvr = ps_pool.tile([P, NC], F32, name="pvr", tag="midps")
                        pvi = ps_pool.tile([P, NC], F32, name="pvi", tag="midps")
                        for db in range(n_dblk):
                            nc.tensor.matmul(pvr,
                                             lhsT=w_tiles["wvr"][db][:, ts(eb, P)],
                                             rhs=xts[db], start=(db == 0),
                                             stop=(db == n_dblk - 1))
                        for j in range(n_pair):
                            nc.tensor.matmul(pvi,
                                             lhsT=wvi_tiles[j][:, :, ts(eb, P)],
                                             rhs=x8s[j], perf_mode=DR,
                                             start=(j == 0),
                                             stop=(j == n_pair - 1))
                        # gate = c0 + c1*tanh(a1 x + b1) + c2*tanh(a2 x + b2);
                        # pvi holds VI*SW*SX so fold 1/(SW*SX) into the scales
                        t1 = gate_pool.tile([P, NC], BF16, name="t1", tag="t1")
                        nc.scalar.activation(out=t1, in_=pvi, func=AF.Tanh,
                                             scale=GA1 / VI_SCL, bias=gb1)
                        t2 = gate_pool.tile([P, NC], BF16, name="t2", tag="t2")
                        nc.scalar.activation(out=t2, in_=pvi, func=AF.Tanh,
                                             scale=GA2 / VI_SCL, bias=gb2)
                        g2 = gate_pool.tile([P, NC], BF16, name="g2", tag="g2")
                        nc.vector.scalar_tensor_tensor(out=g2, in0=t2,
                                                       scalar=GC2 / GC1,
                                                       in1=t1, op0=ALU.mult,
                                                       op1=ALU.add)
                        g3 = gate_pool.tile([P, NC], BF16, name="g3", tag="g3")
                        nc.vector.tensor_scalar(out=g3, in0=g2, scalar1=GC1,
                                                scalar2=GC0, op0=ALU.mult,
                                                op1=ALU.add)
                        vt = vt_pool.tile([P, NC], BF16, name=f"vt{eb}",
                                          tag=f"vt{eb}")
                        nc.vector.tensor_mul(out=vt, in0=g3, in1=pvr)
                        vts.append(vt)
                    return vts

                # wvi8/wq8 stream in under the Tk matmuls; first needed by
                # VRVI(0) / Uq at the end of the mid phase (wvr spreads
                # across the G loop below)
                for j in range(n_pair):
                    for i in range(2):
                        nc.sync.dma_start(
                            out=wvi_tiles[j][:, i, :],
                            in_=wvi8[2 * j * P + i * P:2 * j * P + (i + 1) * P, :])
                        nc.scalar.dma_start(
                            out=wq8_tiles[j][:, i, :],
                            in_=wq8[2 * j * P + i * P:2 * j * P + (i + 1) * P, :])

                # ---------------- Mid: Tk, Uq, G, norms, A' ----------------
                with tc.tile_pool(name="mid_ps", bufs=5, space="PSUM") as mid_ps, \
                     tc.tile_pool(name="nrm_ps", bufs=1, space="PSUM") as nrm_ps:

                    # Tk = S @ WkT ; nk2 = colsum(WkT * Tk) in transpose-
                    # readiness order. The W*T products accumulate on the DVE
                    # in bf16 (partition sums commute across ib blocks), so
                    # the partition-reduce is one matmul per 128-block,
                    # emitted directly in COLUMN layout.
                    nkc_ps = nrm_ps.tile([P, n_dblk], F32, name="nkc", tag="nkc")
                    nq2_ps = {}
                    # the S->fp8 pair casts for Uq ride these iterations
                    for ib_pos, ib in enumerate([4, 5, 2, 3, 0, 1][:n_dblk]):
                        for e in range(n_ef):
                            pt = mid_ps.tile([P, ef], F32, name="pt", tag="midps")
                            tk_unit(ib, e, pt, first=False,
                                    tmp_pool=wtmp_pool)
                            # S -> fp8 pair casts (scaled 2^-5), one per (ib,e)
                            ci = ib_pos * n_ef + e
                            if 0 <= ci < 2 * n_pair:
                                j, i = divmod(ci, 2)
                                if ci % 2 == 0:
                                    nc.vector.tensor_scalar_mul(
                                        out=s8_tiles[j][:, i, :],
                                        in0=sb_tiles[2 * j + i], scalar1=SXS)
                                else:
                                    nc.scalar.activation(
                                        out=s8_tiles[j][:, i, :],
                                        in_=sb_tiles[2 * j + i], func=AF.Copy,
                                        scale=SXS)

                    # Uq = S @ WqT in fp8 DoubleRow (Uq feeds only the norm
                    # nq2 = colsum(WqT * Uq) — a sum of squares, so the ~4%
                    # fp8 noise averages to ~0.1% on rq); psum carries
                    # Uq * UQ_SCL, divided out in the rsqrt activation
                    for e in range(n_ef):
                        for ib in range(n_dblk):
                            pt = mid_ps.tile([P, ef], F32, name="pu", tag="midps")
                            for j in range(n_pair):
                                nc.tensor.matmul(pt,
                                                 lhsT=s8_tiles[j][:, :, ts(ib, P)],
                                                 rhs=wq8_tiles[j][:, :, ts(e, ef)],
                                                 perf_mode=DR, start=(j == 0),
                                                 stop=(j == n_pair - 1))
                            tmp = wtmp_pool.tile([P, ef], BF16, name="tmq", tag="wtmpb")
                            nc.vector.tensor_mul(out=tmp, in0=pt,
                                                 in1=w_tiles["wq"][ib][:, ts(e, ef)])
                            if ib == 0:
                                nc.vector.tensor_copy(out=acc_q[e], in_=tmp)
                            else:
                                nc.vector.tensor_add(out=acc_q[e], in0=acc_q[e], in1=tmp)
                            if e == 0 and ib == 1:
                                # nk2 partition-reduce straight into column
                                # layout [P, n_dblk]: one tiny matmul per
                                # 128-block; single bank, one start=True
                                for db in range(n_dblk):
                                    ee, dd = divmod(db, n_dblk // n_ef)
                                    nc.tensor.matmul(
                                        nkc_ps[:, db:db + 1],
                                        lhsT=acc_k[ee][:, ts(dd, P)],
                                        rhs=onesb, start=(db == 0), stop=True)
                            if e == 0 and ib == 3:
                                # rk = 1/sqrt(nk2) on ACT, column form (the
                                # +1e-5 in the reference is a 1.6e-7 relative
                                # effect at these norms)
                                nc.scalar.activation(out=rk_col, in_=nkc_ps,
                                                     func=AF.Abs_reciprocal_sqrt)
                            if e == 0 and ib == 4:
                                nc.vector.tensor_scalar_mul(out=rk625, in0=rk_col,
                                                            scalar1=0.625)
                                nc.vector.tensor_scalar_mul(out=rk375, in0=rk_col,
                                                            scalar1=0.375)
                    # G = Tk^T @ WqT ; A' = rk * (0.625 G + 0.375 |G|) from PSUM.
                    # The trailing nq2 ones-matmul, the rq chain and the first
                    # VRVI chunk all interleave with the G groups so the PE
                    # never idles across the mid->pass2 transition.
                    g_groups = [(eb, e) for eb in range(n_dblk) for e in range(n_ef)]
                    for gi, (eb, e) in enumerate(g_groups):
                        if gi % 2 == 0 and gi // 2 < n_dblk:
                            # wvr trickles in across the G phase; first needed
                            # by VRVI(0) right after it
                            db = gi // 2
                            eng = nc.sync if db % 2 == 0 else nc.scalar
                            eng.dma_start(out=w_tiles["wvr"][db],
                                          in_=wvr[ts(db, P), :])
                        pg = mid_ps.tile([P, ef], F32, name="pg", tag="midps")
                        for db in range(n_dblk):
                            nc.tensor.matmul(pg, lhsT=tkb_tiles[db][:, ts(eb, P)],
                                             rhs=w_tiles["wq"][db][:, ts(e, ef)],
                                             start=(db == 0), stop=(db == n_dblk - 1))
                        if gi == 0:
                            for ee in range(n_ef):
                                nq2_ps[ee] = nrm_ps.tile([1, ef], F32,
                                                         name=f"nq2{ee}",
                                                         tag=f"nrm{ee}")
                                nc.tensor.matmul(nq2_ps[ee], lhsT=onesb,
                                                 rhs=acc_q[ee],
                                                 start=True, stop=True)
                        if gi == 2:
                            # rq = 1/sqrt(nq2) on ACT under the G matmuls;
                            # scale removes the fp8 operand scaling (UQ_SCL)
                            for ee in range(n_ef):
                                nc.scalar.activation(out=nq[0:1, ts(ee, ef)],
                                                     in_=nq2_ps[ee],
                                                     func=AF.Abs_reciprocal_sqrt,
                                                     scale=1.0 / UQ_SCL)
                        if gi == 5:
                            for ee in range(n_ef):
                                pb = mid_ps.tile([P, ef], F32, name="pb", tag="midps")
                                nc.tensor.matmul(pb, lhsT=ones_row,
                                                 rhs=nq[0:1, ts(ee, ef)],
                                                 start=True, stop=True)
                                nc.vector.tensor_copy(out=rq_bc[:, ts(ee, ef)], in_=pb)
                        tabs = wtmp_pool.tile([P, ef], F32, name="tabs", tag="wtmp")
                        nc.scalar.activation(out=tabs, in_=pg, func=AF.Abs,
                                             scale=rk375[:, eb:eb + 1])
                        nc.vector.scalar_tensor_tensor(
                            out=a_tiles[eb][:, ts(e, ef)], in0=pg,
                            scalar=rk625[:, eb:eb + 1], in1=tabs,
                            op0=ALU.mult, op1=ALU.add)

                    def issue_out(c, vts, ps_pool):
                        for s in range(n_sub):
                            for e in range(n_ef):
                                po = ps_pool.tile([P, ef], F32, name="po", tag="midps")
                                for eb in range(n_dblk):
                                    nc.tensor.matmul(po, lhsT=vts[eb][:, ts(s, P)],
                                                     rhs=a_tiles[eb][:, ts(e, ef)],
                                                     start=(eb == 0),
                                                     stop=(eb == n_dblk - 1))
                                ot = osb_pool.tile([P, ef], BF16, name="ot", tag="osb")
                                nc.vector.tensor_mul(out=ot, in0=po,
                                                     in1=rq_bc[:, ts(e, ef)])
                                nc.sync.dma_start(
                                    out=out_d[c * NC + s * P:c * NC + (s + 1) * P,
                                              ts(e, ef)],
                                    in_=ot)

                    # first two VRVI chunks AND the first out phase run out of
                    # the mid psum pool (same tile shape): every later pool-
                    # boundary bank reuse is then separated from its previous
                    # consumer by a full VRVI or out phase of PE work, so the
                    # transition never waits on a trailing gate chain
                    vts_fifo = [issue_vrvi_pool(0, xts_cache.pop(0), mid_ps),
                                issue_vrvi_pool(1, xts_cache.pop(1), mid_ps)]
                    xts_cache[2] = dma_xt(2)
                    issue_out(0, vts_fifo[0], mid_ps)

                # ---------------- Pass 2: VRVI / out pipeline ----------------
                with tc.tile_pool(name="vrvi_ps", bufs=4, space="PSUM") as vrvi_ps, \
                     tc.tile_pool(name="out_ps", bufs=4, space="PSUM") as out_ps:

                    for c in range(1, n_chunks):
                        if c + 1 < n_chunks:
                            vts_fifo.append(
                                issue_vrvi_pool(c + 1, xts_cache.pop(c + 1), vrvi_ps))
                        if c + 2 < n_chunks:
                            xts_cache[c + 2] = dma_xt(c + 2)
                        issue_out(c, vts_fifo[c], out_ps)
    nc.compile()
    return nc


_PROGRAM_CACHE = {}


def _get_program(n, d):
    key = (n, d)
    if key not in _PROGRAM_CACHE:
        _PROGRAM_CACHE[key] = build_program(n, d)
    return _PROGRAM_CACHE[key]


def _numpy_reference(x, Wvr, bvr, Wvi, bvi, Wk, bk, Wq, bq):
    """Slow fp32 fallback (never expected to run: biases are zeros)."""
    out = np.empty_like(x)
    for b in range(x.shape[0]):
        xb = x[b].astype(np.float64)
        vr = xb @ Wvr.T.astype(np.float64) + bvr
        vi = xb @ Wvi.T.astype(np.float64) + bvi
        v = vr * np.tanh(np.logaddexp(0.0, vi))
        k = xb @ Wk.T.astype(np.float64) + bk
        q = xb @ Wq.T.astype(np.float64) + bq
        kn = k / (np.linalg.norm(k, axis=0, keepdims=True) + 1e-5)
        qn = q / (np.linalg.norm(q, axis=0, keepdims=True) + 1e-5)
        g = kn.T @ qn
        a = 0.625 * g + 0.375 * np.abs(g)
        out[b] = (v @ a).astype(np.float32)
    return out


def kernel(_run_kwargs=None, **inputs):
    run_kwargs = _run_kwargs or {}
    x = np.asarray(inputs["x"], dtype=np.float32)
    Wvr = np.asarray(inputs["Wvr"], dtype=np.float32)
    Wvi = np.asarray(inputs["Wvi"], dtype=np.float32)
    Wk = np.asarray(inputs["Wk"], dtype=np.float32)
    Wq = np.asarray(inputs["Wq"], dtype=np.float32)
    bvr, bvi = np.asarray(inputs["bvr"]), np.asarray(inputs["bvi"])
    bk, bq = np.asarray(inputs["bk"]), np.asarray(inputs["bq"])

    if any(np.any(b != 0) for b in (bvr, bvi, bk, bq)):
        return _numpy_reference(x, Wvr, bvr, Wvi, bvi, Wk, bk, Wq, bq)

    b, n, d = x.shape
    assert b == B and n == N_FULL and d == D_FULL, (b, n, d)

    bf16 = ml_dtypes.bfloat16
    fp8 = ml_dtypes.float8_e4m3
    wvr_t = np.ascontiguousarray(Wvr.T).astype(bf16)
    wk_t = np.ascontiguousarray(Wk.T).astype(bf16)
    wq_t = np.ascontiguousarray(Wq.T).astype(bf16)
    wvi8 = np.clip(np.ascontiguousarray(Wvi.T) * SW, -240.0, 240.0).astype(fp8)
    wq8_h = np.clip(np.ascontiguousarray(Wq.T) * SW, -240.0, 240.0).astype(fp8)

    ident = np.eye(P, dtype=bf16)
    in_maps = []
    for i in range(N_CORES):
        xti = np.ascontiguousarray(x[i].T)
        in_maps.append({
            "xn": x[i].astype(bf16),
            "xt": xti.astype(bf16),
            "xt8": np.clip(xti * SX, -240.0, 240.0).astype(fp8),
            "wvr": wvr_t, "wvi8": wvi8, "wk": wk_t, "wq": wq_t,
            "wq8": wq8_h, "ident": ident,
        })

    nc = _get_program(n, d)
    from concourse.bass_utils import run_bass_kernel_spmd
    res = run_bass_kernel_spmd(nc, in_maps, core_ids=list(range(N_CORES)), **run_kwargs)
    out = np.stack([np.asarray(res.results[i]["out"]).astype(np.float32)
                    for i in range(N_CORES)], axis=0)
    if run_kwargs:
        kernel.last_results = res
    return out
